# revision 10
# baseline (speedup 1.0000x reference)
"""Distributed Trainium2 kernel for nn_Encoder_88502096101469.

8-core SPMD layout (one NEFF, per-core data):
- Activations live TRANSPOSED in SBUF as batch-halves: X^T_b (512 feat x
  256 cols) where cols = batch-b rows [256c, 256c+256) for core c.
- Core c owns attention head h=c for BOTH batches. The torch-faithful
  "raw reshape" of (b, h, t, dv) -> (b, t, h*dv) maps head h's output to
  Z rows [256h, 256h+256) per batch, which is exactly core c's resident
  row range -> no post-attention exchange needed.
- Per batch, one 8-way AllToAll exchanges Q^T/K^T slices (64 head rows x
  256 local cols, bf16) and V natural slices (256 rows x 64 head cols).
- The whole layer is pipelined by batch-half: projections, pack, A2A,
  O-projection and LayerNorm of one half overlap attention of the other.
- Projections/LN matmuls run in float32r (TF32-class, 1 cyc/row);
  the attention path (Q/K/V/E) is bf16 (separate LDWEIGHTS pipelining).
- Softmax skips max-subtraction (logits are O(1)); the denominator comes
  from a ones-column appended to V (lhsT M=65); exp folds the 1/8 scale.
- LayerNorm stats (feature axis = partitions) via ones-vector matmuls;
  rstd = exp(-0.5*ln(var+eps)) keeps ACT on one table set (no reloads).
"""
import numpy as np
import ml_dtypes

import concourse.bass as bass
import concourse.bacc as bacc
import concourse.tile as tile
from concourse import mybir
from concourse import bass_utils

NCORES = 8
DIM = 512
HALF = 256          # per-core cols per batch
NITER = 3           # LAYERS + 1
LN_EPS = 1e-5

F32 = mybir.dt.float32
F32R = mybir.dt.float32r
BF16 = mybir.dt.bfloat16
I32 = mybir.dt.int32
I16 = mybir.dt.int16
AF = mybir.ActivationFunctionType
OP = mybir.AluOpType

# Schraudolph exp on DVE: bf16(2^t) bits ~= i16(t*128 + 128*(127-sigma)),
# computed as one f32 tensor_scalar (mult, add) with i16 output dtype.
# t = raw_score * 0.125 * log2(e).
_LOG2E = 1.4426950408889634
SCH_A = 0.125 * _LOG2E * 128.0
SCH_SIGMA = 0.0430
SCH_B = 128.0 * (127.0 - SCH_SIGMA)

# A2A per-batch shard layout (flat bf16 words per (src,dst) pair):
#   [0:16384)      Q^T slice  (64 of-rows, 256 cols)
#   [16384:32768)  K^T slice  (64 of-rows, 256 cols)
#   [32768:49152)  V slice    (2 t-chunks, 128 rows, 64 fv-cols)
SHARD = 49152


def _build_graph(nc):
    xt_in = nc.dram_tensor("xt", [DIM, 2 * HALF], F32R, kind="ExternalInput").ap()
    wq_in = nc.dram_tensor("wq", [DIM, DIM], F32R, kind="ExternalInput").ap()
    wk_in = nc.dram_tensor("wk", [DIM, DIM], F32R, kind="ExternalInput").ap()
    wv_in = nc.dram_tensor("wv", [DIM, DIM], F32R, kind="ExternalInput").ap()
    wo_in = nc.dram_tensor("wo", [DIM, DIM], F32R, kind="ExternalInput").ap()
    bq_in = nc.dram_tensor("bq", [128, 4], F32, kind="ExternalInput").ap()
    bk_in = nc.dram_tensor("bk", [128, 4], F32, kind="ExternalInput").ap()
    bo_in = nc.dram_tensor("bo", [128, 4], F32, kind="ExternalInput").ap()
    bv_in = nc.dram_tensor("bv", [1, DIM], F32R, kind="ExternalInput").ap()
    lng_in = nc.dram_tensor("lng", [128, 4], F32, kind="ExternalInput").ap()
    lnb_in = nc.dram_tensor("lnb", [128, 4], F32, kind="ExternalInput").ap()
    ones_in = nc.dram_tensor("ones", [128, 128], F32R, kind="ExternalInput").ap()
    ones3_in = nc.dram_tensor("ones3", [128, 16, 1], BF16, kind="ExternalInput").ap()
    out_d = nc.dram_tensor("out", [DIM, 2 * HALF], F32R, kind="ExternalOutput").ap()

    groups = [list(range(NCORES))]

    from contextlib import ExitStack
    with tile.TileContext(nc) as tc, ExitStack() as ctx:
        const = ctx.enter_context(tc.tile_pool(name="const", bufs=1))
        act = ctx.enter_context(tc.tile_pool(name="act", bufs=1))
        qkv = ctx.enter_context(tc.tile_pool(name="qkv", bufs=1))
        gath = ctx.enter_context(tc.tile_pool(name="gath", bufs=2))
        epool = ctx.enter_context(tc.tile_pool(name="epool", bufs=3))
        small = ctx.enter_context(tc.tile_pool(name="small", bufs=1))
        dram = ctx.enter_context(tc.tile_pool(name="dram", bufs=1, space="DRAM"))
        s_psum = ctx.enter_context(tc.tile_pool(name="s_psum", bufs=2, space="PSUM"))
        o_psum = ctx.enter_context(tc.tile_pool(name="o_psum", bufs=2, space="PSUM"))
        p_psum = ctx.enter_context(tc.tile_pool(name="p_psum", bufs=1, space="PSUM"))
        ln_psum = ctx.enter_context(tc.tile_pool(name="ln_psum", bufs=1, space="PSUM"))
        if True:
            # ---- constants to SBUF ----
            def load_w(ap_in, nm):
                t = const.tile([128, 4, DIM], F32R, name=nm, tag=nm)
                nc.sync.dma_start(out=t, in_=ap_in.rearrange("(c p) f -> p c f", p=128))
                return t

            wq, wk, wv, wo = (load_w(wq_in, "wqt"), load_w(wk_in, "wkt"),
                              load_w(wv_in, "wvt"), load_w(wo_in, "wot"))
            bq = const.tile([128, 4], F32)
            bk = const.tile([128, 4], F32)
            bo = const.tile([128, 4], F32)
            lng = const.tile([128, 4], F32)
            lnb = const.tile([128, 4], F32)
            for t, a in ((bq, bq_in), (bk, bk_in), (bo, bo_in), (lng, lng_in), (lnb, lnb_in)):
                nc.sync.dma_start(out=t, in_=a)
            bv = const.tile([1, DIM], F32R)
            nc.sync.dma_start(out=bv, in_=bv_in)
            ones = const.tile([128, 128], F32R)
            nc.sync.dma_start(out=ones, in_=ones_in)
            ones3 = const.tile([128, 16, 1], BF16)
            nc.sync.dma_start(out=ones3, in_=ones3_in)
            eps_t = const.tile([1, 1], F32)
            nc.vector.memset(eps_t, LN_EPS)
            magic = const.tile([1, HALF], I32)
            nc.vector.memset(magic, 0x5F3759DF)

            # initial activation, as halves
            x0h = []
            for b in range(2):
                xb = act.tile([128, 4, HALF], F32R, tag=f"x0h{b}", name=f"x0h{b}")
                nc.sync.dma_start(
                    out=xb,
                    in_=xt_in.rearrange("(c p) f -> p c f", p=128)[:, :, 256 * b:256 * (b + 1)])
                x0h.append(xb)

            # DRAM bounce buffers
            sendb = [dram.tile([NCORES, SHARD], BF16, tag=f"send{b}",
                               name=f"send{b}") for b in range(2)]
            recvb = [dram.tile([NCORES, SHARD], BF16, tag=f"recv{b}",
                               name=f"recv{b}") for b in range(2)]
            rs_d = dram.tile([1, 512], F32, tag="rs_d", name="rs_d", bufs=2)
            stat_d = [dram.tile([2, HALF], F32, tag=f"stat{b}",
                                name=f"stat{b}", bufs=2) for b in range(2)]

            def proj_T_half(x_b, w, btile, tag, odt=F32R):
                """(128,4,HALF) <- relu(w^T x_b + bias), transposed output."""
                out = qkv.tile([128, 4, HALF], odt, tag=tag, name=tag)
                for pair in range(2):
                    ps = p_psum.tile([128, 2, HALF], F32, tag="p", name=f"ps_{tag}")
                    for i in range(2):
                        ofc = 2 * pair + i
                        for ifc in range(4):
                            nc.tensor.matmul(
                                ps[:, i, :],
                                w[:, ifc, 128 * ofc:128 * (ofc + 1)],
                                x_b[:, ifc, :],
                                start=(ifc == 0), stop=(ifc == 3))
                        nc.vector.tensor_scalar(
                            out=out[:, ofc, :], in0=ps[:, i, :],
                            scalar1=btile[:, ofc:ofc + 1], scalar2=0.0,
                            op0=OP.add, op1=OP.max)
                return out

            def proj_V_half(x_b, tag):
                """(128,2,DIM) bf16 <- relu(x_b^T wv + bv), natural layout."""
                out = qkv.tile([128, 2, DIM], BF16, tag=tag, name=tag)
                for tch in range(2):
                    ps = p_psum.tile([128, DIM], F32, tag="p", name=f"ps_{tag}{tch}")
                    for ifc in range(4):
                        nc.tensor.matmul(
                            ps,
                            x_b[:, ifc, 128 * tch:128 * (tch + 1)],
                            wv[:, ifc, :],
                            start=(ifc == 0), stop=False)
                    nc.tensor.matmul(
                        ps, ones[0:1, :], bv, start=False, stop=True)
                    nc.vector.tensor_scalar(
                        out=out[:, tch, :], in0=ps,
                        scalar1=0.0, scalar2=None, op0=OP.max)
                return out

            def exchange_half(b, qt_b, kt_b, v_b):
                sb, rb = sendb[b], recvb[b]
                for d in range(NCORES):
                    cq, p0 = d // 2, 64 * (d % 2)
                    nc.sync.dma_start(
                        out=sb[d, 0:16384].rearrange("(r c) -> r c", c=256),
                        in_=qt_b[p0:p0 + 64, cq, :])
                    nc.sync.dma_start(
                        out=sb[d, 16384:32768].rearrange("(r c) -> r c", c=256),
                        in_=kt_b[p0:p0 + 64, cq, :])
                    nc.sync.dma_start(
                        out=sb[d, 32768:49152].rearrange(
                            "(tc p j) -> p tc j", tc=2, p=128),
                        in_=v_b[:, :, 64 * d:64 * (d + 1)])
                nc.gpsimd.collective_compute(
                    "AllToAll", OP.bypass, replica_groups=groups,
                    ins=[sb.opt()], outs=[rb.opt()])
                # qh: q^T features on partitions 0-63, duplicated on 64-127 so
                # row-tile T8 can stream its own copy.  kh: even kt-chunks on
                # partitions 0-63 (tile T0), odd kt-chunks on 64-127 (tile T8).
                qh = gath.tile([128, NCORES, 256], BF16, tag=f"qh{b}", name=f"qh{b}")
                kh = gath.tile([128, NCORES, 128], BF16, tag=f"kh{b}", name=f"kh{b}")
                vh = gath.tile([128, 16, 65], BF16, tag=f"vh{b}", name=f"vh{b}")
                qsrc = rb[:, 0:16384].rearrange("s (r c) -> r s c", r=64)
                nc.sync.dma_start(out=qh[0:64], in_=qsrc)
                nc.sync.dma_start(out=qh[64:128], in_=qsrc)
                ksrc = rb[:, 16384:32768].rearrange("s (r c) -> r s c", r=64)
                nc.sync.dma_start(out=kh[0:64], in_=ksrc[:, :, 0:128])
                nc.sync.dma_start(out=kh[64:128], in_=ksrc[:, :, 128:256])
                for tc2 in range(2):
                    nc.sync.dma_start(
                        out=vh[:, tc2::2, 0:64],
                        in_=rb[:, 32768 + 8192 * tc2:32768 + 8192 * (tc2 + 1)]
                            .rearrange("s (p j) -> p s j", p=128))
                nc.sync.dma_start(out=vh[:, :, 64:65], in_=ones3)
                return qh, kh, vh

            def attention_half(b, qh, kh, vh):
                """(128,4,HALF) f32r Z^T for batch b (local Z rows)."""
                z = qkv.tile([128, 4, HALF], F32R, tag=f"z{b}", name=f"z{b}")

                def norm_a(ops, j):
                    # rsum copy on ACT (PSUM-near), recip + broadcast kicked off
                    rsum = small.tile([1, 512], F32, tag="rsum", bufs=2, name="rsum")
                    nc.scalar.activation(rsum, ops[64:65, :], AF.Copy)
                    recip = small.tile([1, 512], F32, tag="recip", bufs=2,
                                       name="recip")
                    nc.vector.reciprocal_approx_fast(recip, rsum)
                    nc.sync.dma_start(out=rs_d, in_=recip)
                    rrep = small.tile([64, 512], F32, tag="rrep", bufs=2,
                                      name="rrep")
                    nc.sync.dma_start(
                        out=rrep, in_=rs_d.partition_broadcast(64)[:, 0, :])
                    return rrep

                def norm_b(ops, j, rrep):
                    o_v = ops[0:64, :].rearrange("f (r s) -> f s r", s=8)
                    r_v = rrep.rearrange("f (r s) -> f s r", s=8)
                    for q in range(2):
                        nc.vector.tensor_tensor(
                            out=z[64 * q:64 * (q + 1), :, 64 * j:64 * (j + 1)],
                            in0=o_v[:, q::2, :],
                            in1=r_v[:, q::2, :],
                            op=OP.mult)

                # j-pipelined: normalization of chunk j-1 is emitted inside
                # chunk j's p-loop so its DRAM-bounce broadcast hides behind
                # the exp stream instead of head-of-line blocking the DVE.
                pend = None
                for j in range(4):
                    ops = o_psum.tile([65, 512], F32, tag="o", name=f"ops{b}{j}")

                    def emit_av(ep, pp, last):
                        nc.tensor.matmul(ops, vh[:, 2 * pp, :], ep[:, 0, :],
                                         start=(pp == 0), stop=False)
                        nc.tensor.matmul(ops, vh[:, 2 * pp + 1, :], ep[:, 1, :],
                                         start=False, stop=last)

                    # software-pipelined: AV for step p-1 is emitted AFTER the
                    # scores of step p, so the PE FIFO never stalls on exp.
                    prev = None
                    for p in range(8):
                        sps = s_psum.tile([128, 2, 512], F32, tag="s",
                                          name=f"sps{b}{j}{p}")
                        # concurrent row tiles: T0 (even chunk), T8 (odd chunk)
                        nc.tensor.matmul(
                            sps[:, 0, :], kh[0:64, p, :],
                            qh[0:64, 2 * j:2 * j + 2, :], start=True, stop=True)
                        nc.tensor.matmul(
                            sps[:, 1, :], kh[64:128, p, :],
                            qh[64:128, 2 * j:2 * j + 2, :], start=True, stop=True)
                        if prev is not None:
                            emit_av(*prev, last=False)
                        e = epool.tile([128, 2, 512], BF16, tag="e", name=f"e{b}{j}{p}")
                        # exp split: ACT on bank 0, DVE schraudolph on bank 1
                        nc.scalar.activation(e[:, 0, :], sps[:, 0, :],
                                             AF.Exp, scale=0.125)
                        nc.vector.tensor_scalar(
                            out=e.bitcast(I16)[:, 1, :], in0=sps[:, 1, :],
                            scalar1=SCH_A, scalar2=SCH_B,
                            op0=OP.mult, op1=OP.add)
                        prev = (e, p)
                        if p == 1 and pend is not None:
                            pend = pend + (norm_a(*pend),)
                    emit_av(*prev, last=True)
                    if pend is not None:
                        norm_b(*pend)
                    pend = (ops, j)
                pend = pend + (norm_a(*pend),)
                norm_b(*pend)
                return z

            def ln_half(x_b, b, resid=None):
                """LN over features (partitions) on one batch-half."""
                if resid is not None:
                    xr = act.tile([128, 4, HALF], F32R, tag=f"xr{b}", name=f"xr{b}")
                    nc.vector.tensor_tensor(out=xr, in0=x_b, in1=resid, op=OP.add)
                    x_b = xr
                x2 = act.tile([128, 4, HALF], F32R, tag=f"x2{b}", name=f"x2{b}")
                nc.vector.tensor_tensor(out=x2, in0=x_b, in1=x_b, op=OP.mult)
                mu_ps = ln_psum.tile([1, HALF], F32, tag="ln", name=f"lnmu{b}")
                for ifc in range(4):
                    nc.tensor.matmul(mu_ps, ones[:, 0:1], x_b[:, ifc, :],
                                     start=(ifc == 0), stop=(ifc == 3))
                mu = small.tile([1, HALF], F32, tag="mu", bufs=2, name="mu")
                nc.vector.tensor_scalar(out=mu, in0=mu_ps, scalar1=1.0 / DIM,
                                        scalar2=None, op0=OP.mult)
                m2_ps = ln_psum.tile([1, HALF], F32, tag="ln", name=f"lnm2{b}")
                for ifc in range(4):
                    nc.tensor.matmul(m2_ps, ones[:, 1:2], x2[:, ifc, :],
                                     start=(ifc == 0), stop=(ifc == 3))
                ex2 = small.tile([1, HALF], F32, tag="ex2", bufs=2, name="ex2")
                nc.vector.tensor_scalar(out=ex2, in0=m2_ps, scalar1=1.0 / DIM,
                                        scalar2=None, op0=OP.mult)
                m2 = small.tile([1, HALF], F32, tag="m2", bufs=2, name="m2")
                nc.vector.tensor_tensor(out=m2, in0=mu, in1=mu, op=OP.mult)
                nc.vector.tensor_tensor(out=ex2, in0=ex2, in1=m2, op=OP.subtract)
                nc.vector.tensor_scalar(out=ex2, in0=ex2, scalar1=LN_EPS,
                                        scalar2=None, op0=OP.add)
                # rstd = rsqrt(var+eps) fully on DVE: bit-trick seed + 2 Newton
                # steps (keeps ACT on the exp table set -> no table reloads).
                sd = small.tile([1, HALF], F32, tag="sd", bufs=2, name="sd")
                sdi = sd.bitcast(I32)
                nc.vector.tensor_scalar(out=sdi, in0=ex2.bitcast(I32), scalar1=1,
                                        scalar2=None, op0=OP.logical_shift_right)
                nc.vector.tensor_tensor(out=sdi, in0=magic, in1=sdi, op=OP.subtract)
                for _ in range(2):
                    nc.vector.tensor_tensor(out=m2, in0=ex2, in1=sd, op=OP.mult)
                    nc.vector.tensor_tensor(out=m2, in0=m2, in1=sd, op=OP.mult)
                    nc.vector.tensor_scalar(out=m2, in0=m2, scalar1=-0.5,
                                            scalar2=1.5, op0=OP.mult, op1=OP.add)
                    nc.vector.tensor_tensor(out=sd, in0=sd, in1=m2, op=OP.mult)
                nc.vector.tensor_tensor(out=mu, in0=mu, in1=sd, op=OP.mult)
                # broadcast rstd & mu*rstd across partitions via DRAM bounce
                nc.sync.dma_start(out=stat_d[b][0:1, :], in_=sd)
                nc.sync.dma_start(out=stat_d[b][1:2, :], in_=mu)
                srep = small.tile([128, 2, HALF], F32, tag=f"srep{b}", bufs=2,
                                  name=f"srep{b}")
                nc.sync.dma_start(out=srep, in_=stat_d[b].partition_broadcast(128))
                out = act.tile([128, 4, HALF], F32R, tag=f"lnout{b}", bufs=3,
                               name=f"lnout{b}")
                for ifc in range(4):
                    t1 = small.tile([128, HALF], F32, tag=f"t1{b}", bufs=2,
                                    name=f"t1{b}")
                    nc.vector.tensor_tensor(out=t1, in0=x_b[:, ifc, :],
                                            in1=srep[:, 0, :], op=OP.mult)
                    nc.vector.tensor_tensor(out=t1, in0=t1, in1=srep[:, 1, :],
                                            op=OP.subtract)
                    nc.vector.tensor_scalar(
                        out=out[:, ifc, :], in0=t1,
                        scalar1=lng[:, ifc:ifc + 1], scalar2=lnb[:, ifc:ifc + 1],
                        op0=OP.mult, op1=OP.add)
                return out

            def exchange(b, xb):
                qt_b = proj_T_half(xb, wq, bq, f"qt{b}", odt=BF16)
                kt_b = proj_T_half(xb, wk, bk, f"kt{b}", odt=BF16)
                v_b = proj_V_half(xb, f"v{b}")
                return exchange_half(b, qt_b, kt_b, v_b)

            # Staged pipeline over sublayers: for each batch, the NEXT
            # sublayer's qkv projection + pack + A2A is emitted right after
            # this batch's LN, so the collective flies while the other
            # batch's attention keeps the PE dense.
            NSUB = 2 * NITER
            xin = x0h
            prev_in = None
            pend = [exchange(b, x0h[b]) for b in range(2)]
            for s in range(NSUB):
                nxt = [None, None]
                for b in range(2):
                    z_b = attention_half(b, *pend[b])
                    y_b = proj_T_half(z_b, wo, bo, f"y{b}", odt=F32R)
                    resid = prev_in[b] if (s % 2 == 1) else None
                    nxt[b] = ln_half(y_b, b, resid=resid)
                    if s < NSUB - 1:
                        pend[b] = exchange(b, nxt[b])
                prev_in = xin
                xin = nxt
            init = xin

            for b in range(2):
                nc.sync.dma_start(
                    out=out_d.rearrange("(c p) f -> p c f", p=128)[:, :, 256 * b:256 * (b + 1)],
                    in_=init[b])
    return nc


_NC_CACHE = None


def _get_nc():
    global _NC_CACHE
    if _NC_CACHE is None:
        nc = bacc.Bacc("TRN2", target_bir_lowering=False, debug=False,
                       num_devices=NCORES)
        _build_graph(nc)
        nc.compile()
        _NC_CACHE = nc
    return _NC_CACHE


def kernel(encoder_inputs, Wq, bq, Wk, bk, Wv, bv, Wo, bo, ln_g, ln_b,
           _trace=False, _trace_kwargs=None):
    x = np.asarray(encoder_inputs, dtype=np.float32)
    consts = {
        "wq": np.ascontiguousarray(np.asarray(Wq, np.float32)),
        "wk": np.ascontiguousarray(np.asarray(Wk, np.float32)),
        "wv": np.ascontiguousarray(np.asarray(Wv, np.float32)),
        "wo": np.ascontiguousarray(np.asarray(Wo, np.float32)),
        "bq": np.ascontiguousarray(np.asarray(bq, np.float32).reshape(4, 128).T),
        "bk": np.ascontiguousarray(np.asarray(bk, np.float32).reshape(4, 128).T),
        "bo": np.ascontiguousarray(np.asarray(bo, np.float32).reshape(4, 128).T),
        "bv": np.asarray(bv, np.float32).reshape(1, DIM),
        "lng": np.ascontiguousarray(np.asarray(ln_g, np.float32).reshape(4, 128).T),
        "lnb": np.ascontiguousarray(np.asarray(ln_b, np.float32).reshape(4, 128).T),
        "ones": np.ones((128, 128), np.float32),
        "ones3": np.ones((128, 16, 1), ml_dtypes.bfloat16),
    }
    in_maps = []
    for c in range(NCORES):
        xt = np.concatenate([x[0, 256 * c:256 * (c + 1)].T,
                             x[1, 256 * c:256 * (c + 1)].T], axis=1)
        in_maps.append({"xt": np.ascontiguousarray(xt), **consts})

    nc = _get_nc()
    res = bass_utils.run_bass_kernel_spmd(
        nc, in_maps, core_ids=list(range(NCORES)),
        trace=_trace, **(_trace_kwargs or {}))

    out = np.zeros((2, 2048, DIM), np.float32)
    for c in range(NCORES):
        r = res.results[c]["out"]
        out[0, 256 * c:256 * (c + 1)] = r[:, :256].T
        out[1, 256 * c:256 * (c + 1)] = r[:, 256:].T
    if _trace:
        kernel._last_results = res
    return out



# revision 15
# speedup vs baseline: 1.1919x; 1.1919x over previous
"""Distributed Trainium2 kernel for nn_Encoder_88502096101469.

8-core SPMD layout (one NEFF, per-core data):
- Activations live TRANSPOSED in SBUF as batch-halves: X^T_b (512 feat x
  256 cols) where cols = batch-b rows [256c, 256c+256) for core c.
- Core c owns attention head h=c for BOTH batches. The torch-faithful
  "raw reshape" of (b, h, t, dv) -> (b, t, h*dv) maps head h's output to
  Z rows [256h, 256h+256) per batch, which is exactly core c's resident
  row range -> no post-attention exchange needed.
- Per batch, one 8-way AllToAll exchanges Q^T/K^T slices (64 head rows x
  256 local cols, bf16) and V natural slices (256 rows x 64 head cols).
- The whole layer is pipelined by batch-half: projections, pack, A2A,
  O-projection and LayerNorm of one half overlap attention of the other.
- Projections/LN matmuls run in float32r (TF32-class, 1 cyc/row);
  the attention path (Q/K/V/E) is bf16 (separate LDWEIGHTS pipelining).
- Softmax skips max-subtraction (logits are O(1)); the denominator comes
  from a ones-column appended to V (lhsT M=65); exp folds the 1/8 scale.
- LayerNorm stats (feature axis = partitions) via ones-vector matmuls;
  rstd = exp(-0.5*ln(var+eps)) keeps ACT on one table set (no reloads).
"""
import numpy as np
import ml_dtypes
from collections import deque

import concourse.bass as bass
import concourse.bacc as bacc
import concourse.tile as tile
from concourse import mybir
from concourse import bass_utils

NCORES = 8
DIM = 512
HALF = 256          # per-core cols per batch
NITER = 3           # LAYERS + 1
LN_EPS = 1e-5

F32 = mybir.dt.float32
F32R = mybir.dt.float32r
BF16 = mybir.dt.bfloat16
I32 = mybir.dt.int32
I16 = mybir.dt.int16
AF = mybir.ActivationFunctionType
OP = mybir.AluOpType

# Schraudolph exp on DVE: bf16(2^t) bits ~= i16(t*128 + 128*(127-sigma)),
# computed as one f32 tensor_scalar (mult, add) with i16 output dtype.
# t = raw_score * 0.125 * log2(e).
_LOG2E = 1.4426950408889634
SCH_A = 0.125 * _LOG2E * 128.0
SCH_SIGMA = 0.0430
SCH_B = 128.0 * (127.0 - SCH_SIGMA)

# A2A per-batch shard layout (flat bf16 words per (src,dst) pair):
#   [0:16384)      Q^T slice  (64 of-rows, 256 cols)
#   [16384:32768)  K^T slice  (64 of-rows, 256 cols)
#   [32768:49152)  V slice    (2 t-chunks, 128 rows, 64 fv-cols)
SHARD = 49152


def _build_graph(nc):
    xt_in = nc.dram_tensor("xt", [DIM, 2 * HALF], F32R, kind="ExternalInput").ap()
    wq_in = nc.dram_tensor("wq", [DIM, DIM], F32R, kind="ExternalInput").ap()
    wk_in = nc.dram_tensor("wk", [DIM, DIM], F32R, kind="ExternalInput").ap()
    wv_in = nc.dram_tensor("wv", [DIM, DIM], F32R, kind="ExternalInput").ap()
    wo_in = nc.dram_tensor("wo", [DIM, DIM], F32R, kind="ExternalInput").ap()
    bq_in = nc.dram_tensor("bq", [128, 4], F32, kind="ExternalInput").ap()
    bk_in = nc.dram_tensor("bk", [128, 4], F32, kind="ExternalInput").ap()
    bo_in = nc.dram_tensor("bo", [128, 4], F32, kind="ExternalInput").ap()
    bv_in = nc.dram_tensor("bv", [1, DIM], F32R, kind="ExternalInput").ap()
    lng_in = nc.dram_tensor("lng", [128, 4], F32, kind="ExternalInput").ap()
    lnb_in = nc.dram_tensor("lnb", [128, 4], F32, kind="ExternalInput").ap()
    ones_in = nc.dram_tensor("ones", [128, 128], F32R, kind="ExternalInput").ap()
    ones3_in = nc.dram_tensor("ones3", [128, 16, 1], BF16, kind="ExternalInput").ap()
    out_d = nc.dram_tensor("out", [DIM, 2 * HALF], F32R, kind="ExternalOutput").ap()

    groups = [list(range(NCORES))]

    from contextlib import ExitStack
    with tile.TileContext(nc) as tc, ExitStack() as ctx:
        const = ctx.enter_context(tc.tile_pool(name="const", bufs=1))
        act = ctx.enter_context(tc.tile_pool(name="act", bufs=1))
        qkv = ctx.enter_context(tc.tile_pool(name="qkv", bufs=1))
        gath = ctx.enter_context(tc.tile_pool(name="gath", bufs=2))
        epool = ctx.enter_context(tc.tile_pool(name="epool", bufs=3))
        small = ctx.enter_context(tc.tile_pool(name="small", bufs=1))
        dram = ctx.enter_context(tc.tile_pool(name="dram", bufs=1, space="DRAM"))
        s_psum = ctx.enter_context(tc.tile_pool(name="s_psum", bufs=2, space="PSUM"))
        o_psum = ctx.enter_context(tc.tile_pool(name="o_psum", bufs=2, space="PSUM"))
        p_psum = ctx.enter_context(tc.tile_pool(name="p_psum", bufs=1, space="PSUM"))
        ln_psum = ctx.enter_context(tc.tile_pool(name="ln_psum", bufs=1, space="PSUM"))
        if True:
            # ---- constants to SBUF ----
            def load_w(ap_in, nm):
                t = const.tile([128, 4, DIM], F32R, name=nm, tag=nm)
                nc.sync.dma_start(out=t, in_=ap_in.rearrange("(c p) f -> p c f", p=128))
                return t

            wq, wk, wv, wo = (load_w(wq_in, "wqt"), load_w(wk_in, "wkt"),
                              load_w(wv_in, "wvt"), load_w(wo_in, "wot"))
            bq = const.tile([128, 4], F32)
            bk = const.tile([128, 4], F32)
            bo = const.tile([128, 4], F32)
            lng = const.tile([128, 4], F32)
            lnb = const.tile([128, 4], F32)
            for t, a in ((bq, bq_in), (bk, bk_in), (bo, bo_in), (lng, lng_in), (lnb, lnb_in)):
                nc.sync.dma_start(out=t, in_=a)
            bv = const.tile([1, DIM], F32R)
            nc.sync.dma_start(out=bv, in_=bv_in)
            ones = const.tile([128, 128], F32R)
            nc.sync.dma_start(out=ones, in_=ones_in)
            ones3 = const.tile([128, 16, 1], BF16)
            nc.sync.dma_start(out=ones3, in_=ones3_in)
            eps_t = const.tile([1, 1], F32)
            nc.vector.memset(eps_t, LN_EPS)
            magic = const.tile([1, HALF], I32)
            nc.vector.memset(magic, 0x5F3759DF)

            # initial activation, as halves
            x0h = []
            for b in range(2):
                xb = act.tile([128, 4, HALF], F32R, tag=f"x0h{b}", name=f"x0h{b}")
                nc.sync.dma_start(
                    out=xb,
                    in_=xt_in.rearrange("(c p) f -> p c f", p=128)[:, :, 256 * b:256 * (b + 1)])
                x0h.append(xb)

            # DRAM bounce buffers
            sendb = [dram.tile([NCORES, SHARD], BF16, tag=f"send{b}",
                               name=f"send{b}") for b in range(2)]
            recvb = [dram.tile([NCORES, SHARD], BF16, tag=f"recv{b}",
                               name=f"recv{b}") for b in range(2)]
            rs_d = dram.tile([1, 512], F32, tag="rs_d", name="rs_d", bufs=2)
            stat_d = [dram.tile([2, HALF], F32, tag=f"stat{b}",
                                name=f"stat{b}", bufs=2) for b in range(2)]

            def proj_T_half(x_b, w, btile, tag, odt=F32R):
                """(128,4,HALF) <- relu(w^T x_b + bias), transposed output."""
                out = qkv.tile([128, 4, HALF], odt, tag=tag, name=tag)
                for pair in range(2):
                    ps = p_psum.tile([128, 2, HALF], F32, tag="p", name=f"ps_{tag}")
                    for i in range(2):
                        ofc = 2 * pair + i
                        for ifc in range(4):
                            nc.tensor.matmul(
                                ps[:, i, :],
                                w[:, ifc, 128 * ofc:128 * (ofc + 1)],
                                x_b[:, ifc, :],
                                start=(ifc == 0), stop=(ifc == 3))
                        nc.vector.tensor_scalar(
                            out=out[:, ofc, :], in0=ps[:, i, :],
                            scalar1=btile[:, ofc:ofc + 1], scalar2=0.0,
                            op0=OP.add, op1=OP.max)
                return out

            def proj_V_half(x_b, tag):
                """(128,2,DIM) bf16 <- relu(x_b^T wv + bv), natural layout."""
                out = qkv.tile([128, 2, DIM], BF16, tag=tag, name=tag)
                for tch in range(2):
                    ps = p_psum.tile([128, DIM], F32, tag="p", name=f"ps_{tag}{tch}")
                    for ifc in range(4):
                        nc.tensor.matmul(
                            ps,
                            x_b[:, ifc, 128 * tch:128 * (tch + 1)],
                            wv[:, ifc, :],
                            start=(ifc == 0), stop=False)
                    nc.tensor.matmul(
                        ps, ones[0:1, :], bv, start=False, stop=True)
                    nc.vector.tensor_scalar(
                        out=out[:, tch, :], in0=ps,
                        scalar1=0.0, scalar2=None, op0=OP.max)
                return out

            def make_exchange(b, xb):
                """qkv projection + pack + A2A + unpack as a list of small
                emission pieces, to be interleaved into the following
                attention so no engine FIFO gets a long head-of-line block.
                Tiles are allocated eagerly; instructions emit when pieces
                are invoked (in list order)."""
                qt_b = qkv.tile([128, 4, HALF], BF16, tag=f"qt{b}", name=f"qt{b}")
                kt_b = qkv.tile([128, 4, HALF], BF16, tag=f"kt{b}", name=f"kt{b}")
                v_b = qkv.tile([128, 2, DIM], BF16, tag=f"v{b}", name=f"v{b}")
                # qh: q^T features on partitions 0-63, duplicated on 64-127 so
                # row-tile T8 can stream its own copy.  kh: even kt-chunks on
                # partitions 0-63 (tile T0), odd kt-chunks on 64-127 (tile T8).
                qh = gath.tile([128, NCORES, 256], BF16, tag=f"qh{b}", name=f"qh{b}")
                kh = gath.tile([128, NCORES, 128], BF16, tag=f"kh{b}", name=f"kh{b}")
                vh = gath.tile([128, 16, 65], BF16, tag=f"vh{b}", name=f"vh{b}")
                sb, rb = sendb[b], recvb[b]
                pieces = []

                def qk_piece(w, btile, out, pair, tag):
                    def f():
                        ps = p_psum.tile([128, 2, HALF], F32, tag="p",
                                         name=f"ps_{tag}{pair}")
                        for i in range(2):
                            ofc = 2 * pair + i
                            for ifc in range(4):
                                nc.tensor.matmul(
                                    ps[:, i, :],
                                    w[:, ifc, 128 * ofc:128 * (ofc + 1)],
                                    xb[:, ifc, :],
                                    start=(ifc == 0), stop=(ifc == 3))
                            nc.vector.tensor_scalar(
                                out=out[:, ofc, :], in0=ps[:, i, :],
                                scalar1=btile[:, ofc:ofc + 1], scalar2=0.0,
                                op0=OP.add, op1=OP.max)
                    return f

                def v_piece(tch):
                    def f():
                        ps = p_psum.tile([128, DIM], F32, tag="p",
                                         name=f"ps_v{b}{tch}")
                        for ifc in range(4):
                            nc.tensor.matmul(
                                ps,
                                xb[:, ifc, 128 * tch:128 * (tch + 1)],
                                wv[:, ifc, :],
                                start=(ifc == 0), stop=False)
                        nc.tensor.matmul(
                            ps, ones[0:1, :], bv, start=False, stop=True)
                        nc.vector.tensor_scalar(
                            out=v_b[:, tch, :], in0=ps,
                            scalar1=0.0, scalar2=None, op0=OP.max)
                    return f

                def pack_piece(dlo, dhi):
                    def f():
                        for d in range(dlo, dhi):
                            cq, p0 = d // 2, 64 * (d % 2)
                            nc.gpsimd.dma_start(
                                out=sb[d, 0:16384].rearrange("(r c) -> r c", c=256),
                                in_=qt_b[p0:p0 + 64, cq, :])
                            nc.gpsimd.dma_start(
                                out=sb[d, 16384:32768].rearrange("(r c) -> r c", c=256),
                                in_=kt_b[p0:p0 + 64, cq, :])
                            nc.gpsimd.dma_start(
                                out=sb[d, 32768:49152].rearrange(
                                    "(tc p j) -> p tc j", tc=2, p=128),
                                in_=v_b[:, :, 64 * d:64 * (d + 1)])
                    return f

                def coll_piece():
                    nc.gpsimd.collective_compute(
                        "AllToAll", OP.bypass, replica_groups=groups,
                        ins=[sb.opt()], outs=[rb.opt()])

                def unpack_piece():
                    qsrc = rb[:, 0:16384].rearrange("s (r c) -> r s c", r=64)
                    nc.gpsimd.dma_start(out=qh[0:64], in_=qsrc)
                    nc.gpsimd.dma_start(out=qh[64:128], in_=qsrc)
                    ksrc = rb[:, 16384:32768].rearrange("s (r c) -> r s c", r=64)
                    nc.gpsimd.dma_start(out=kh[0:64], in_=ksrc[:, :, 0:128])
                    nc.gpsimd.dma_start(out=kh[64:128], in_=ksrc[:, :, 128:256])
                    for tc2 in range(2):
                        nc.gpsimd.dma_start(
                            out=vh[:, tc2::2, 0:64],
                            in_=rb[:, 32768 + 8192 * tc2:32768 + 8192 * (tc2 + 1)]
                                .rearrange("s (p j) -> p s j", p=128))
                    nc.gpsimd.dma_start(out=vh[:, :, 64:65], in_=ones3)

                pieces += [qk_piece(wq, bq, qt_b, pr, f"qt{b}") for pr in range(2)]
                pieces += [qk_piece(wk, bk, kt_b, pr, f"kt{b}") for pr in range(2)]
                pieces += [v_piece(tch) for tch in range(2)]
                pieces += [pack_piece(0, 4), pack_piece(4, 8),
                           coll_piece, unpack_piece]
                return pieces, (qh, kh, vh)

            def attention_half(b, qh, kh, vh, pieces=None):
                """(128,4,HALF) f32r Z^T for batch b (local Z rows).
                `pieces`: deque of emission closures (the next exchange)
                interleaved into the p-loop, ~one every other p-step."""
                pieces = pieces if pieces is not None else deque()
                z = qkv.tile([128, 4, HALF], F32R, tag=f"z{b}", name=f"z{b}")

                def norm_a(ops, j):
                    # rsum copy on ACT (PSUM-near), recip + broadcast kicked off
                    rsum = small.tile([1, 512], F32, tag="rsum", bufs=2, name="rsum")
                    nc.scalar.activation(rsum, ops[64:65, :], AF.Copy)
                    recip = small.tile([1, 512], F32, tag="recip", bufs=2,
                                       name="recip")
                    nc.vector.reciprocal_approx_fast(recip, rsum)
                    nc.sync.dma_start(out=rs_d, in_=recip)
                    rrep = small.tile([64, 512], F32, tag="rrep", bufs=2,
                                      name="rrep")
                    nc.sync.dma_start(
                        out=rrep, in_=rs_d.partition_broadcast(64)[:, 0, :])
                    return rrep

                def norm_b(ops, j, rrep):
                    o_v = ops[0:64, :].rearrange("f (r s) -> f s r", s=8)
                    r_v = rrep.rearrange("f (r s) -> f s r", s=8)
                    for q in range(2):
                        nc.vector.tensor_tensor(
                            out=z[64 * q:64 * (q + 1), :, 64 * j:64 * (j + 1)],
                            in0=o_v[:, q::2, :],
                            in1=r_v[:, q::2, :],
                            op=OP.mult)

                # j-pipelined: normalization of chunk j-1 is emitted inside
                # chunk j's p-loop so its DRAM-bounce broadcast hides behind
                # the exp stream instead of head-of-line blocking the DVE.
                pend = None
                for j in range(4):
                    ops = o_psum.tile([65, 512], F32, tag="o", name=f"ops{b}{j}")

                    def emit_av(ep, pp, last):
                        nc.tensor.matmul(ops, vh[:, 2 * pp, :], ep[:, 0, :],
                                         start=(pp == 0), stop=False)
                        nc.tensor.matmul(ops, vh[:, 2 * pp + 1, :], ep[:, 1, :],
                                         start=False, stop=last)

                    # software-pipelined: AV for step p-1 is emitted AFTER the
                    # scores of step p, so the PE FIFO never stalls on exp.
                    prev = None
                    for p in range(8):
                        sps = s_psum.tile([128, 2, 512], F32, tag="s",
                                          name=f"sps{b}{j}{p}")
                        # concurrent row tiles: T0 (even chunk), T8 (odd chunk)
                        nc.tensor.matmul(
                            sps[:, 0, :], kh[0:64, p, :],
                            qh[0:64, 2 * j:2 * j + 2, :], start=True, stop=True)
                        nc.tensor.matmul(
                            sps[:, 1, :], kh[64:128, p, :],
                            qh[64:128, 2 * j:2 * j + 2, :], start=True, stop=True)
                        if prev is not None:
                            emit_av(*prev, last=False)
                        e = epool.tile([128, 2, 512], BF16, tag="e", name=f"e{b}{j}{p}")
                        # exp at pair granularity, alternating engines: one
                        # instruction covers both banks; ACT on even p, DVE
                        # schraudolph (i16 bit-trick) on odd p.  Each engine
                        # gets two p-step periods per call -> latency slack.
                        if p % 2 == 0:
                            nc.scalar.activation(e, sps, AF.Exp, scale=0.125)
                        else:
                            nc.vector.tensor_scalar(
                                out=e.bitcast(I16), in0=sps,
                                scalar1=SCH_A, scalar2=SCH_B,
                                op0=OP.mult, op1=OP.add)
                        prev = (e, p)
                        if p == 1 and pend is not None:
                            pend = pend + (norm_a(*pend),)
                        if p % 2 == 1 and pieces:
                            pieces.popleft()()
                    emit_av(*prev, last=True)
                    if pend is not None:
                        norm_b(*pend)
                    pend = (ops, j)
                pend = pend + (norm_a(*pend),)
                norm_b(*pend)
                return z

            def ln_half(x_b, b, resid=None):
                """LN over features (partitions) on one batch-half."""
                if resid is not None:
                    xr = act.tile([128, 4, HALF], F32R, tag=f"xr{b}", name=f"xr{b}")
                    nc.vector.tensor_tensor(out=xr, in0=x_b, in1=resid, op=OP.add)
                    x_b = xr
                x2 = act.tile([128, 4, HALF], F32R, tag=f"x2{b}", name=f"x2{b}")
                nc.vector.tensor_tensor(out=x2, in0=x_b, in1=x_b, op=OP.mult)
                mu_ps = ln_psum.tile([1, HALF], F32, tag="ln", name=f"lnmu{b}")
                for ifc in range(4):
                    nc.tensor.matmul(mu_ps, ones[:, 0:1], x_b[:, ifc, :],
                                     start=(ifc == 0), stop=(ifc == 3))
                mu = small.tile([1, HALF], F32, tag="mu", bufs=2, name="mu")
                nc.vector.tensor_scalar(out=mu, in0=mu_ps, scalar1=1.0 / DIM,
                                        scalar2=None, op0=OP.mult)
                m2_ps = ln_psum.tile([1, HALF], F32, tag="ln", name=f"lnm2{b}")
                for ifc in range(4):
                    nc.tensor.matmul(m2_ps, ones[:, 1:2], x2[:, ifc, :],
                                     start=(ifc == 0), stop=(ifc == 3))
                ex2 = small.tile([1, HALF], F32, tag="ex2", bufs=2, name="ex2")
                nc.vector.tensor_scalar(out=ex2, in0=m2_ps, scalar1=1.0 / DIM,
                                        scalar2=None, op0=OP.mult)
                m2 = small.tile([1, HALF], F32, tag="m2", bufs=2, name="m2")
                nc.vector.tensor_tensor(out=m2, in0=mu, in1=mu, op=OP.mult)
                nc.vector.tensor_tensor(out=ex2, in0=ex2, in1=m2, op=OP.subtract)
                nc.vector.tensor_scalar(out=ex2, in0=ex2, scalar1=LN_EPS,
                                        scalar2=None, op0=OP.add)
                # rstd = rsqrt(var+eps) fully on DVE: bit-trick seed + 2 Newton
                # steps (keeps ACT on the exp table set -> no table reloads).
                sd = small.tile([1, HALF], F32, tag="sd", bufs=2, name="sd")
                sdi = sd.bitcast(I32)
                nc.vector.tensor_scalar(out=sdi, in0=ex2.bitcast(I32), scalar1=1,
                                        scalar2=None, op0=OP.logical_shift_right)
                nc.vector.tensor_tensor(out=sdi, in0=magic, in1=sdi, op=OP.subtract)
                for _ in range(2):
                    nc.vector.tensor_tensor(out=m2, in0=ex2, in1=sd, op=OP.mult)
                    nc.vector.tensor_tensor(out=m2, in0=m2, in1=sd, op=OP.mult)
                    nc.vector.tensor_scalar(out=m2, in0=m2, scalar1=-0.5,
                                            scalar2=1.5, op0=OP.mult, op1=OP.add)
                    nc.vector.tensor_tensor(out=sd, in0=sd, in1=m2, op=OP.mult)
                nc.vector.tensor_tensor(out=mu, in0=mu, in1=sd, op=OP.mult)
                # broadcast rstd & mu*rstd across partitions via DRAM bounce
                nc.sync.dma_start(out=stat_d[b][0:1, :], in_=sd)
                nc.sync.dma_start(out=stat_d[b][1:2, :], in_=mu)
                srep = small.tile([128, 2, HALF], F32, tag=f"srep{b}", bufs=2,
                                  name=f"srep{b}")
                nc.sync.dma_start(out=srep, in_=stat_d[b].partition_broadcast(128))
                out = act.tile([128, 4, HALF], F32R, tag=f"lnout{b}", bufs=3,
                               name=f"lnout{b}")
                for ifc in range(4):
                    t1 = small.tile([128, HALF], F32, tag=f"t1{b}", bufs=2,
                                    name=f"t1{b}")
                    nc.vector.tensor_tensor(out=t1, in0=x_b[:, ifc, :],
                                            in1=srep[:, 0, :], op=OP.mult)
                    nc.vector.tensor_tensor(out=t1, in0=t1, in1=srep[:, 1, :],
                                            op=OP.subtract)
                    nc.vector.tensor_scalar(
                        out=out[:, ifc, :], in0=t1,
                        scalar1=lng[:, ifc:ifc + 1], scalar2=lnb[:, ifc:ifc + 1],
                        op0=OP.mult, op1=OP.add)
                return out

            # Staged pipeline over (sublayer, batch) stages.  The exchange
            # (qkv proj + pack + A2A + unpack) for stage (s+1, b) is created
            # right after stage (s, b)'s LN and its pieces are interleaved
            # into the FOLLOWING stage's attention p-loop, so no engine FIFO
            # ever sees a multi-microsecond head-of-line block and the
            # collective flies under attention compute.
            NSUB = 2 * NITER
            cur_in = list(x0h)
            prev_in = [None, None]
            pk0, h0 = make_exchange(0, x0h[0])
            for f in pk0:
                f()
            pk1, h1 = make_exchange(1, x0h[1])
            for f in pk1:
                f()
            pend = [h0, h1]
            carry = deque()
            for s in range(NSUB):
                for b in range(2):
                    z_b = attention_half(b, *pend[b], pieces=carry)
                    while carry:
                        carry.popleft()()
                    y_b = proj_T_half(z_b, wo, bo, f"y{b}", odt=F32R)
                    resid = prev_in[b] if (s % 2 == 1) else None
                    xout = ln_half(y_b, b, resid=resid)
                    if s < NSUB - 1:
                        carry, pend[b] = make_exchange(b, xout)
                        carry = deque(carry)
                    else:
                        carry = deque()
                    prev_in[b] = cur_in[b]
                    cur_in[b] = xout
            init = cur_in

            for b in range(2):
                nc.sync.dma_start(
                    out=out_d.rearrange("(c p) f -> p c f", p=128)[:, :, 256 * b:256 * (b + 1)],
                    in_=init[b])
    return nc


_NC_CACHE = None


def _get_nc():
    global _NC_CACHE
    if _NC_CACHE is None:
        nc = bacc.Bacc("TRN2", target_bir_lowering=False, debug=False,
                       num_devices=NCORES)
        _build_graph(nc)
        nc.compile()
        _NC_CACHE = nc
    return _NC_CACHE


def kernel(encoder_inputs, Wq, bq, Wk, bk, Wv, bv, Wo, bo, ln_g, ln_b,
           _trace=False, _trace_kwargs=None):
    x = np.asarray(encoder_inputs, dtype=np.float32)
    consts = {
        "wq": np.ascontiguousarray(np.asarray(Wq, np.float32)),
        "wk": np.ascontiguousarray(np.asarray(Wk, np.float32)),
        "wv": np.ascontiguousarray(np.asarray(Wv, np.float32)),
        "wo": np.ascontiguousarray(np.asarray(Wo, np.float32)),
        "bq": np.ascontiguousarray(np.asarray(bq, np.float32).reshape(4, 128).T),
        "bk": np.ascontiguousarray(np.asarray(bk, np.float32).reshape(4, 128).T),
        "bo": np.ascontiguousarray(np.asarray(bo, np.float32).reshape(4, 128).T),
        "bv": np.asarray(bv, np.float32).reshape(1, DIM),
        "lng": np.ascontiguousarray(np.asarray(ln_g, np.float32).reshape(4, 128).T),
        "lnb": np.ascontiguousarray(np.asarray(ln_b, np.float32).reshape(4, 128).T),
        "ones": np.ones((128, 128), np.float32),
        "ones3": np.ones((128, 16, 1), ml_dtypes.bfloat16),
    }
    in_maps = []
    for c in range(NCORES):
        xt = np.concatenate([x[0, 256 * c:256 * (c + 1)].T,
                             x[1, 256 * c:256 * (c + 1)].T], axis=1)
        in_maps.append({"xt": np.ascontiguousarray(xt), **consts})

    nc = _get_nc()
    res = bass_utils.run_bass_kernel_spmd(
        nc, in_maps, core_ids=list(range(NCORES)),
        trace=_trace, **(_trace_kwargs or {}))

    out = np.zeros((2, 2048, DIM), np.float32)
    for c in range(NCORES):
        r = res.results[c]["out"]
        out[0, 256 * c:256 * (c + 1)] = r[:, :256].T
        out[1, 256 * c:256 * (c + 1)] = r[:, 256:].T
    if _trace:
        kernel._last_results = res
    return out



# revision 28
# speedup vs baseline: 1.3327x; 1.1181x over previous
"""Distributed Trainium2 kernel for nn_Encoder_88502096101469.

8-core SPMD layout (one NEFF, per-core data):
- Activations live TRANSPOSED in SBUF as batch-halves: X^T_b (512 feat x
  256 cols) where cols = batch-b rows [256c, 256c+256) for core c.
- Core c owns attention head h=c for BOTH batches. The torch-faithful
  "raw reshape" of (b, h, t, dv) -> (b, t, h*dv) maps head h's output to
  Z rows [256h, 256h+256) per batch, which is exactly core c's resident
  row range -> no post-attention exchange needed.
- Per batch, one 8-way AllToAll exchanges Q^T/K^T slices (64 head rows x
  256 local cols, bf16) and V natural slices (256 rows x 64 head cols).
- The whole layer is pipelined by batch-half: projections, pack, A2A,
  O-projection and LayerNorm of one half overlap attention of the other.
- Projections/LN matmuls run in float32r (TF32-class, 1 cyc/row);
  the attention path (Q/K/V/E) is bf16 (separate LDWEIGHTS pipelining).
- Softmax skips max-subtraction (logits are O(1)); the denominator comes
  from a ones-column appended to V (lhsT M=65); exp folds the 1/8 scale.
- LayerNorm stats (feature axis = partitions) via ones-vector matmuls;
  rstd = exp(-0.5*ln(var+eps)) keeps ACT on one table set (no reloads).
"""
import numpy as np
import ml_dtypes
from collections import deque

import concourse.bass as bass
import concourse.bacc as bacc
import concourse.tile as tile
from concourse import mybir
from concourse import bass_utils

NCORES = 8
DIM = 512
HALF = 256          # per-core cols per batch
NITER = 3           # LAYERS + 1
LN_EPS = 1e-5

F32 = mybir.dt.float32
F32R = mybir.dt.float32r
BF16 = mybir.dt.bfloat16
I32 = mybir.dt.int32
I16 = mybir.dt.int16
AF = mybir.ActivationFunctionType
OP = mybir.AluOpType

# Schraudolph exp on DVE: bf16(2^t) bits ~= i16(t*128 + 128*(127-sigma)),
# computed as one f32 tensor_scalar (mult, add) with i16 output dtype.
# t = raw_score * 0.125 * log2(e).
_LOG2E = 1.4426950408889634
SCH_A = 0.125 * _LOG2E * 128.0
SCH_SIGMA = 0.0430
SCH_B = 128.0 * (127.0 - SCH_SIGMA)

# A2A per-batch shard layout (flat bf16 words per (src,dst) pair):
#   [0:16384)      Q^T slice  (64 of-rows, 256 cols)
#   [16384:32768)  K^T slice  (64 of-rows, 256 cols)
#   [32768:49152)  V slice    (2 t-chunks, 128 rows, 64 fv-cols)
SHARD = 49152


def _build_graph(nc):
    xt_in = nc.dram_tensor("xt", [DIM, 2 * HALF], F32R, kind="ExternalInput").ap()
    wq_in = nc.dram_tensor("wq", [DIM, DIM], F32R, kind="ExternalInput").ap()
    wk_in = nc.dram_tensor("wk", [DIM, DIM], F32R, kind="ExternalInput").ap()
    wv_in = nc.dram_tensor("wv", [DIM, DIM], F32R, kind="ExternalInput").ap()
    wo_in = nc.dram_tensor("wo", [DIM, DIM], F32R, kind="ExternalInput").ap()
    bq_in = nc.dram_tensor("bq", [128, 4], F32, kind="ExternalInput").ap()
    bk_in = nc.dram_tensor("bk", [128, 4], F32, kind="ExternalInput").ap()
    bo_in = nc.dram_tensor("bo", [128, 4], F32, kind="ExternalInput").ap()
    bv_in = nc.dram_tensor("bv", [1, DIM], F32R, kind="ExternalInput").ap()
    lng_in = nc.dram_tensor("lng", [128, 4], F32, kind="ExternalInput").ap()
    lnb_in = nc.dram_tensor("lnb", [128, 4], F32, kind="ExternalInput").ap()
    ones_in = nc.dram_tensor("ones", [128, 128], F32R, kind="ExternalInput").ap()
    ones3_in = nc.dram_tensor("ones3", [128, 16, 1], BF16, kind="ExternalInput").ap()
    out_d = nc.dram_tensor("out", [DIM, 2 * HALF], F32R, kind="ExternalOutput").ap()

    groups = [list(range(NCORES))]

    from contextlib import ExitStack
    with tile.TileContext(nc) as tc, ExitStack() as ctx:
        const = ctx.enter_context(tc.tile_pool(name="const", bufs=1))
        act = ctx.enter_context(tc.tile_pool(name="act", bufs=1))
        qkv = ctx.enter_context(tc.tile_pool(name="qkv", bufs=1))
        gath = ctx.enter_context(tc.tile_pool(name="gath", bufs=2))
        epool = ctx.enter_context(tc.tile_pool(name="epool", bufs=3))
        small = ctx.enter_context(tc.tile_pool(name="small", bufs=1))
        dram = ctx.enter_context(tc.tile_pool(name="dram", bufs=1, space="DRAM"))
        s_psum = ctx.enter_context(tc.tile_pool(name="s_psum", bufs=2, space="PSUM"))
        o_psum = ctx.enter_context(tc.tile_pool(name="o_psum", bufs=2, space="PSUM"))
        p_psum = ctx.enter_context(tc.tile_pool(name="p_psum", bufs=1, space="PSUM"))
        if True:
            # ---- constants to SBUF ----
            def load_w(ap_in, nm):
                t = const.tile([128, 4, DIM], F32R, name=nm, tag=nm)
                nc.sync.dma_start(out=t, in_=ap_in.rearrange("(c p) f -> p c f", p=128))
                return t

            wq, wk, wv, wo = (load_w(wq_in, "wqt"), load_w(wk_in, "wkt"),
                              load_w(wv_in, "wvt"), load_w(wo_in, "wot"))
            bq = const.tile([128, 4], F32)
            bk = const.tile([128, 4], F32)
            bo = const.tile([128, 4], F32)
            lng = const.tile([128, 4], F32)
            lnb = const.tile([128, 4], F32)
            for t, a in ((bq, bq_in), (bk, bk_in), (bo, bo_in), (lng, lng_in), (lnb, lnb_in)):
                nc.sync.dma_start(out=t, in_=a)
            bv = const.tile([1, DIM], F32R)
            nc.sync.dma_start(out=bv, in_=bv_in)
            ones = const.tile([128, 128], F32R)
            nc.sync.dma_start(out=ones, in_=ones_in)
            ones3 = const.tile([128, 16, 1], BF16)
            nc.sync.dma_start(out=ones3, in_=ones3_in)
            eps_t = const.tile([1, 1], F32)
            nc.vector.memset(eps_t, LN_EPS)
            magic = const.tile([1, HALF], I32)
            nc.vector.memset(magic, 0x5F3759DF)

            # initial activation, as halves
            x0h = []
            for b in range(2):
                xb = act.tile([128, 4, HALF], F32R, tag=f"x0h{b}", name=f"x0h{b}")
                nc.sync.dma_start(
                    out=xb,
                    in_=xt_in.rearrange("(c p) f -> p c f", p=128)[:, :, 256 * b:256 * (b + 1)])
                x0h.append(xb)

            # DRAM bounce buffers
            sendb = [dram.tile([NCORES, SHARD], BF16, tag=f"send{b}",
                               name=f"send{b}") for b in range(2)]
            recvb = [dram.tile([NCORES, SHARD], BF16, tag=f"recv{b}",
                               name=f"recv{b}") for b in range(2)]
            rs_d = dram.tile([1, 512], F32, tag="rs_d", name="rs_d", bufs=2)
            stat_d = [dram.tile([2, HALF], F32, tag=f"stat{b}",
                                name=f"stat{b}", bufs=2) for b in range(2)]

            def proj_T_half(x_b, w, btile, tag, odt=F32R):
                """(128,4,HALF) <- relu(w^T x_b + bias), transposed output.
                Evacuation on ACT (relu+bias in one activation) to keep the
                DVE free for exp/LN work."""
                out = qkv.tile([128, 4, HALF], odt, tag=tag, name=tag)
                for pair in range(2):
                    ps = p_psum.tile([128, 2, HALF], F32, tag=f"p{pair}", name=f"ps_{tag}")
                    for i in range(2):
                        ofc = 2 * pair + i
                        for ifc in range(4):
                            nc.tensor.matmul(
                                ps[:, i, :],
                                w[:, ifc, 128 * ofc:128 * (ofc + 1)],
                                x_b[:, ifc, :],
                                start=(ifc == 0), stop=(ifc == 3))
                        nc.scalar.activation(
                            out[:, ofc, :], ps[:, i, :], AF.Relu,
                            bias=btile[:, ofc:ofc + 1])
                return out

            def proj_V_half(x_b, tag):
                """(128,2,DIM) bf16 <- relu(x_b^T wv + bv), natural layout."""
                out = qkv.tile([128, 2, DIM], BF16, tag=tag, name=tag)
                for tch in range(2):
                    ps = p_psum.tile([128, DIM], F32, tag="p", name=f"ps_{tag}{tch}")
                    for ifc in range(4):
                        nc.tensor.matmul(
                            ps,
                            x_b[:, ifc, 128 * tch:128 * (tch + 1)],
                            wv[:, ifc, :],
                            start=(ifc == 0), stop=False)
                    nc.tensor.matmul(
                        ps, ones[0:1, :], bv, start=False, stop=True)
                    nc.vector.tensor_scalar(
                        out=out[:, tch, :], in0=ps,
                        scalar1=0.0, scalar2=None, op0=OP.max)
                return out

            def make_exchange(b, xb):
                """qkv projection + pack + A2A + unpack as a list of small
                emission pieces, to be interleaved into the following
                attention so no engine FIFO gets a long head-of-line block.
                Tiles are allocated eagerly; instructions emit when pieces
                are invoked (in list order)."""
                qt_b = qkv.tile([128, 4, HALF], BF16, tag=f"qt{b}", name=f"qt{b}")
                kt_b = qkv.tile([128, 4, HALF], BF16, tag=f"kt{b}", name=f"kt{b}")
                v_b = qkv.tile([128, 2, DIM], BF16, tag=f"v{b}", name=f"v{b}")
                # qh: q^T features on partitions 0-63, duplicated on 64-127 so
                # row-tile T8 can stream its own copy.  kh: even kt-chunks on
                # partitions 0-63 (tile T0), odd kt-chunks on 64-127 (tile T8).
                qh = gath.tile([128, NCORES, 256], BF16, tag=f"qh{b}", name=f"qh{b}")
                kh = gath.tile([128, NCORES, 128], BF16, tag=f"kh{b}", name=f"kh{b}")
                vh = gath.tile([128, 16, 65], BF16, tag=f"vh{b}", name=f"vh{b}")
                sb, rb = sendb[b], recvb[b]
                pieces = []

                # mm and evac are SEPARATE pieces: the evac (DVE) is emitted
                # ~2 slots after its matmuls so it never head-of-line blocks
                # the exp stream while waiting on the PE.
                def qk_mm(w, out, pair, tag):
                    ps = p_psum.tile([128, 2, HALF], F32, tag=f"p{pair % 2}",
                                     name=f"ps_{tag}{pair}")

                    def f():
                        for i in range(2):
                            ofc = 2 * pair + i
                            for ifc in range(4):
                                nc.tensor.matmul(
                                    ps[:, i, :],
                                    w[:, ifc, 128 * ofc:128 * (ofc + 1)],
                                    xb[:, ifc, :],
                                    start=(ifc == 0), stop=(ifc == 3))
                    return f, ps

                def qk_evac(ps, btile, out, pair):
                    def f():
                        for i in range(2):
                            ofc = 2 * pair + i
                            nc.vector.tensor_scalar(
                                out=out[:, ofc, :], in0=ps[:, i, :],
                                scalar1=btile[:, ofc:ofc + 1], scalar2=0.0,
                                op0=OP.add, op1=OP.max)
                    return f

                def v_mm(tch):
                    ps = p_psum.tile([128, DIM], F32, tag=f"p{tch}",
                                     name=f"ps_v{b}{tch}")

                    def f():
                        for ifc in range(4):
                            nc.tensor.matmul(
                                ps,
                                xb[:, ifc, 128 * tch:128 * (tch + 1)],
                                wv[:, ifc, :],
                                start=(ifc == 0), stop=False)
                        nc.tensor.matmul(
                            ps, ones[0:1, :], bv, start=False, stop=True)
                    return f, ps

                def v_evac(ps, tch):
                    def f():
                        nc.vector.tensor_scalar(
                            out=v_b[:, tch, :], in0=ps,
                            scalar1=0.0, scalar2=None, op0=OP.max)
                    return f

                def pack_piece(which):
                    # dst d = 2*cq + pb owns q/k feature rows [64d, 64d+64) =
                    # ofc chunk cq, partition block pb.  One strided DMA per
                    # (tensor, pb) instead of one per destination.
                    def f():
                        if which == 0:
                            for pb in range(2):
                                nc.gpsimd.dma_start(
                                    out=sb[:, 0:16384].rearrange(
                                        "(cq pb) (r c) -> pb r cq c",
                                        pb=2, r=64, c=256)[pb],
                                    in_=qt_b[64 * pb:64 * pb + 64, :, :])
                        elif which == 1:
                            for pb in range(2):
                                nc.gpsimd.dma_start(
                                    out=sb[:, 16384:32768].rearrange(
                                        "(cq pb) (r c) -> pb r cq c",
                                        pb=2, r=64, c=256)[pb],
                                    in_=kt_b[64 * pb:64 * pb + 64, :, :])
                        else:
                            for tch in range(2):
                                nc.gpsimd.dma_start(
                                    out=sb[:, 32768:49152].rearrange(
                                        "d (tc p j) -> tc p d j",
                                        tc=2, p=128)[tch],
                                    in_=v_b[:, tch, :].rearrange(
                                        "p (d j) -> p d j", d=8))
                    return f

                def coll_piece():
                    nc.gpsimd.collective_compute(
                        "AllToAll", OP.bypass, replica_groups=groups,
                        ins=[sb.opt()], outs=[rb.opt()])

                def unpack_piece():
                    qsrc = rb[:, 0:16384].rearrange("s (r c) -> r s c", r=64)
                    nc.gpsimd.dma_start(out=qh[0:64], in_=qsrc)
                    nc.gpsimd.dma_start(out=qh[64:128], in_=qsrc)
                    ksrc = rb[:, 16384:32768].rearrange("s (r c) -> r s c", r=64)
                    nc.gpsimd.dma_start(out=kh[0:64], in_=ksrc[:, :, 0:128])
                    nc.gpsimd.dma_start(out=kh[64:128], in_=ksrc[:, :, 128:256])
                    for tc2 in range(2):
                        nc.gpsimd.dma_start(
                            out=vh[:, tc2::2, 0:64],
                            in_=rb[:, 32768 + 8192 * tc2:32768 + 8192 * (tc2 + 1)]
                                .rearrange("s (p j) -> p s j", p=128))
                    nc.gpsimd.dma_start(out=vh[:, :, 64:65], in_=ones3)

                q0f, q0ps = qk_mm(wq, qt_b, 0, f"qt{b}")
                q1f, q1ps = qk_mm(wq, qt_b, 1, f"qt{b}")
                k0f, k0ps = qk_mm(wk, kt_b, 0, f"kt{b}")
                k1f, k1ps = qk_mm(wk, kt_b, 1, f"kt{b}")
                v0f, v0ps = v_mm(0)
                v1f, v1ps = v_mm(1)
                pieces += [
                    q0f, q1f,
                    qk_evac(q0ps, bq, qt_b, 0), k0f,
                    qk_evac(q1ps, bq, qt_b, 1), k1f,
                    qk_evac(k0ps, bk, kt_b, 0), v0f,
                    qk_evac(k1ps, bk, kt_b, 1), v1f,
                    v_evac(v0ps, 0), v_evac(v1ps, 1),
                    pack_piece(0), pack_piece(1), pack_piece(2),
                    coll_piece, unpack_piece,
                ]
                return pieces, (qh, kh, vh)

            def attention_half(b, qh, kh, vh, pieces=None):
                """(128,4,HALF) f32r Z^T for batch b (local Z rows).
                `pieces`: deque of emission closures (the next exchange)
                interleaved into the p-loop, ~one every other p-step."""
                pieces = pieces if pieces is not None else deque()
                z = qkv.tile([128, 4, HALF], F32R, tag=f"z{b}", name=f"z{b}")

                def norm_a(ops, j):
                    # rsum copy on ACT (PSUM-near), recip + broadcast kicked off
                    rsum = small.tile([1, 512], F32, tag="rsum", bufs=2, name="rsum")
                    nc.scalar.activation(rsum, ops[64:65, :], AF.Copy)
                    recip = small.tile([1, 512], F32, tag="recip", bufs=2,
                                       name="recip")
                    nc.vector.reciprocal_approx_fast(recip, rsum)
                    nc.sync.dma_start(out=rs_d, in_=recip)
                    rrep = small.tile([64, 512], F32, tag="rrep", bufs=2,
                                      name="rrep")
                    nc.sync.dma_start(
                        out=rrep, in_=rs_d.partition_broadcast(64)[:, 0, :])
                    return rrep

                def norm_b(ops, j, rrep):
                    o_v = ops[0:64, :].rearrange("f (r s) -> f s r", s=8)
                    r_v = rrep.rearrange("f (r s) -> f s r", s=8)
                    for q in range(2):
                        nc.vector.tensor_tensor(
                            out=z[64 * q:64 * (q + 1), :, 64 * j:64 * (j + 1)],
                            in0=o_v[:, q::2, :],
                            in1=r_v[:, q::2, :],
                            op=OP.mult)

                # j-pipelined: normalization of chunk j-1 is emitted inside
                # chunk j's p-loop so its DRAM-bounce broadcast hides behind
                # the exp stream instead of head-of-line blocking the DVE.
                pend = None
                for j in range(4):
                    ops = o_psum.tile([65, 512], F32, tag="o", name=f"ops{b}{j}")

                    def emit_av(ep, pp, last):
                        nc.tensor.matmul(ops, vh[:, 2 * pp, :], ep[:, 0, :],
                                         start=(pp == 0), stop=False)
                        nc.tensor.matmul(ops, vh[:, 2 * pp + 1, :], ep[:, 1, :],
                                         start=False, stop=last)

                    # software-pipelined 2 deep: AV for step p-2 is emitted
                    # AFTER the scores of step p, so the PE FIFO never stalls
                    # on the (pair-granular) exp.
                    prevs = deque()
                    for p in range(8):
                        sps = s_psum.tile([128, 2, 512], F32, tag="s",
                                          name=f"sps{b}{j}{p}")
                        # concurrent row tiles: T0 (even chunk), T8 (odd chunk)
                        nc.tensor.matmul(
                            sps[:, 0, :], kh[0:64, p, :],
                            qh[0:64, 2 * j:2 * j + 2, :], start=True, stop=True)
                        nc.tensor.matmul(
                            sps[:, 1, :], kh[64:128, p, :],
                            qh[64:128, 2 * j:2 * j + 2, :], start=True, stop=True)
                        if len(prevs) == 2:
                            emit_av(*prevs.popleft(), last=False)
                        e = epool.tile([128, 2, 512], BF16, tag="e", name=f"e{b}{j}{p}")
                        # exp at pair granularity, alternating engines: one
                        # instruction covers both banks; ACT on even p, DVE
                        # schraudolph (i16 bit-trick) on odd p.  Each engine
                        # gets two p-step periods per call -> latency slack.
                        if p % 2 == 0:
                            nc.scalar.activation(e, sps, AF.Exp, scale=0.125)
                        else:
                            nc.vector.tensor_scalar(
                                out=e.bitcast(I16), in0=sps,
                                scalar1=SCH_A, scalar2=SCH_B,
                                op0=OP.mult, op1=OP.add)
                        prevs.append((e, p))
                        if p == 1 and pend is not None:
                            pend = pend + (norm_a(*pend),)
                        if p % 2 == 1:
                            for _ in range(2):
                                if pieces:
                                    pieces.popleft()()
                    while prevs:
                        ep, pp = prevs.popleft()
                        emit_av(ep, pp, last=(not prevs))
                    if pend is not None:
                        norm_b(*pend)
                    pend = (ops, j)
                pend = pend + (norm_a(*pend),)
                norm_b(*pend)
                return z

            def ln_half(x_b, b, resid=None):
                """LN over features (partitions) on one batch-half."""
                if resid is not None:
                    xr = act.tile([128, 4, HALF], F32R, tag=f"xr{b}", name=f"xr{b}")
                    nc.vector.tensor_tensor(out=xr, in0=x_b, in1=resid, op=OP.add)
                    x_b = xr
                x2 = act.tile([128, 4, HALF], F32R, tag=f"x2{b}", name=f"x2{b}")
                nc.vector.tensor_tensor(out=x2, in0=x_b, in1=x_b, op=OP.mult)
                mu_ps = p_psum.tile([1, HALF], F32, tag="p0", name=f"lnmu{b}")
                for ifc in range(4):
                    nc.tensor.matmul(mu_ps, ones[:, 0:1], x_b[:, ifc, :],
                                     start=(ifc == 0), stop=(ifc == 3))
                mu = small.tile([1, HALF], F32, tag="mu", bufs=2, name="mu")
                nc.vector.tensor_scalar(out=mu, in0=mu_ps, scalar1=1.0 / DIM,
                                        scalar2=None, op0=OP.mult)
                m2_ps = p_psum.tile([1, HALF], F32, tag="p0", name=f"lnm2{b}")
                for ifc in range(4):
                    nc.tensor.matmul(m2_ps, ones[:, 1:2], x2[:, ifc, :],
                                     start=(ifc == 0), stop=(ifc == 3))
                ex2 = small.tile([1, HALF], F32, tag="ex2", bufs=2, name="ex2")
                nc.vector.tensor_scalar(out=ex2, in0=m2_ps, scalar1=1.0 / DIM,
                                        scalar2=None, op0=OP.mult)
                m2 = small.tile([1, HALF], F32, tag="m2", bufs=2, name="m2")
                nc.vector.tensor_tensor(out=m2, in0=mu, in1=mu, op=OP.mult)
                nc.vector.tensor_tensor(out=ex2, in0=ex2, in1=m2, op=OP.subtract)
                nc.vector.tensor_scalar(out=ex2, in0=ex2, scalar1=LN_EPS,
                                        scalar2=None, op0=OP.add)
                # rstd = rsqrt(var+eps) fully on DVE: bit-trick seed + 2 Newton
                # steps (keeps ACT on the exp table set -> no table reloads).
                sd = small.tile([1, HALF], F32, tag="sd", bufs=2, name="sd")
                sdi = sd.bitcast(I32)
                nc.vector.tensor_scalar(out=sdi, in0=ex2.bitcast(I32), scalar1=1,
                                        scalar2=None, op0=OP.logical_shift_right)
                nc.vector.tensor_tensor(out=sdi, in0=magic, in1=sdi, op=OP.subtract)
                for _ in range(2):
                    nc.vector.tensor_tensor(out=m2, in0=ex2, in1=sd, op=OP.mult)
                    nc.vector.tensor_tensor(out=m2, in0=m2, in1=sd, op=OP.mult)
                    nc.vector.tensor_scalar(out=m2, in0=m2, scalar1=-0.5,
                                            scalar2=1.5, op0=OP.mult, op1=OP.add)
                    nc.vector.tensor_tensor(out=sd, in0=sd, in1=m2, op=OP.mult)
                nc.vector.tensor_tensor(out=mu, in0=mu, in1=sd, op=OP.mult)
                # broadcast rstd & mu*rstd across partitions via DRAM bounce
                nc.sync.dma_start(out=stat_d[b][0:1, :], in_=sd)
                nc.sync.dma_start(out=stat_d[b][1:2, :], in_=mu)
                srep = small.tile([128, 2, HALF], F32, tag=f"srep{b}", bufs=2,
                                  name=f"srep{b}")
                nc.sync.dma_start(out=srep, in_=stat_d[b].partition_broadcast(128))
                out = act.tile([128, 4, HALF], F32R, tag=f"lnout{b}", bufs=3,
                               name=f"lnout{b}")
                for ifc in range(4):
                    t1 = small.tile([128, HALF], F32, tag=f"t1{b}", bufs=2,
                                    name=f"t1{b}")
                    nc.vector.tensor_tensor(out=t1, in0=x_b[:, ifc, :],
                                            in1=srep[:, 0, :], op=OP.mult)
                    nc.vector.tensor_tensor(out=t1, in0=t1, in1=srep[:, 1, :],
                                            op=OP.subtract)
                    nc.vector.tensor_scalar(
                        out=out[:, ifc, :], in0=t1,
                        scalar1=lng[:, ifc:ifc + 1], scalar2=lnb[:, ifc:ifc + 1],
                        op0=OP.mult, op1=OP.add)
                return out

            # Staged pipeline over (sublayer, batch) stages.  The exchange
            # (qkv proj + pack + A2A + unpack) for stage (s+1, b) is created
            # right after stage (s, b)'s LN and its pieces are interleaved
            # into the FOLLOWING stage's attention p-loop, so no engine FIFO
            # ever sees a multi-microsecond head-of-line block and the
            # collective flies under attention compute.
            NSUB = 2 * NITER
            cur_in = list(x0h)
            prev_in = [None, None]
            pk0, h0 = make_exchange(0, x0h[0])
            for f in pk0:
                f()
            pk1, h1 = make_exchange(1, x0h[1])
            for f in pk1:
                f()
            pend = [h0, h1]
            carry = deque()
            for s in range(NSUB):
                for b in range(2):
                    z_b = attention_half(b, *pend[b], pieces=carry)
                    while carry:
                        carry.popleft()()
                    y_b = proj_T_half(z_b, wo, bo, f"y{b}", odt=F32R)
                    resid = prev_in[b] if (s % 2 == 1) else None
                    xout = ln_half(y_b, b, resid=resid)
                    if s < NSUB - 1:
                        carry, pend[b] = make_exchange(b, xout)
                        carry = deque(carry)
                    else:
                        carry = deque()
                    prev_in[b] = cur_in[b]
                    cur_in[b] = xout
            init = cur_in

            for b in range(2):
                nc.sync.dma_start(
                    out=out_d.rearrange("(c p) f -> p c f", p=128)[:, :, 256 * b:256 * (b + 1)],
                    in_=init[b])
    return nc


_NC_CACHE = None


def _get_nc():
    global _NC_CACHE
    if _NC_CACHE is None:
        nc = bacc.Bacc("TRN2", target_bir_lowering=False, debug=False,
                       num_devices=NCORES)
        _build_graph(nc)
        nc.compile()
        _NC_CACHE = nc
    return _NC_CACHE


def kernel(encoder_inputs, Wq, bq, Wk, bk, Wv, bv, Wo, bo, ln_g, ln_b,
           _trace=False, _trace_kwargs=None):
    x = np.asarray(encoder_inputs, dtype=np.float32)
    consts = {
        "wq": np.ascontiguousarray(np.asarray(Wq, np.float32)),
        "wk": np.ascontiguousarray(np.asarray(Wk, np.float32)),
        "wv": np.ascontiguousarray(np.asarray(Wv, np.float32)),
        "wo": np.ascontiguousarray(np.asarray(Wo, np.float32)),
        "bq": np.ascontiguousarray(np.asarray(bq, np.float32).reshape(4, 128).T),
        "bk": np.ascontiguousarray(np.asarray(bk, np.float32).reshape(4, 128).T),
        "bo": np.ascontiguousarray(np.asarray(bo, np.float32).reshape(4, 128).T),
        "bv": np.asarray(bv, np.float32).reshape(1, DIM),
        "lng": np.ascontiguousarray(np.asarray(ln_g, np.float32).reshape(4, 128).T),
        "lnb": np.ascontiguousarray(np.asarray(ln_b, np.float32).reshape(4, 128).T),
        "ones": np.ones((128, 128), np.float32),
        "ones3": np.ones((128, 16, 1), ml_dtypes.bfloat16),
    }
    in_maps = []
    for c in range(NCORES):
        xt = np.concatenate([x[0, 256 * c:256 * (c + 1)].T,
                             x[1, 256 * c:256 * (c + 1)].T], axis=1)
        in_maps.append({"xt": np.ascontiguousarray(xt), **consts})

    nc = _get_nc()
    res = bass_utils.run_bass_kernel_spmd(
        nc, in_maps, core_ids=list(range(NCORES)),
        trace=_trace, **(_trace_kwargs or {}))

    out = np.zeros((2, 2048, DIM), np.float32)
    for c in range(NCORES):
        r = res.results[c]["out"]
        out[0, 256 * c:256 * (c + 1)] = r[:, :256].T
        out[1, 256 * c:256 * (c + 1)] = r[:, 256:].T
    if _trace:
        kernel._last_results = res
    return out



# revision 35
# speedup vs baseline: 1.3681x; 1.0265x over previous
"""Distributed Trainium2 kernel for nn_Encoder_88502096101469.

8-core SPMD layout (one NEFF, per-core data):
- Activations live TRANSPOSED in SBUF as batch-halves: X^T_b (512 feat x
  256 cols) where cols = batch-b rows [256c, 256c+256) for core c.
- Core c owns attention head h=c for BOTH batches. The torch-faithful
  "raw reshape" of (b, h, t, dv) -> (b, t, h*dv) maps head h's output to
  Z rows [256h, 256h+256) per batch, which is exactly core c's resident
  row range -> no post-attention exchange needed.
- Per batch, one 8-way AllToAll exchanges Q^T/K^T slices (64 head rows x
  256 local cols, bf16) and V natural slices (256 rows x 64 head cols).
- The whole layer is pipelined by batch-half: projections, pack, A2A,
  O-projection and LayerNorm of one half overlap attention of the other.
- Projections/LN matmuls run in float32r (TF32-class, 1 cyc/row);
  the attention path (Q/K/V/E) is bf16 (separate LDWEIGHTS pipelining).
- Softmax skips max-subtraction (logits are O(1)); the denominator comes
  from a ones-column appended to V (lhsT M=65); exp folds the 1/8 scale.
- LayerNorm stats (feature axis = partitions) via ones-vector matmuls;
  rstd = exp(-0.5*ln(var+eps)) keeps ACT on one table set (no reloads).
"""
import numpy as np
import ml_dtypes
from collections import deque

import concourse.bass as bass
import concourse.bacc as bacc
import concourse.tile as tile
from concourse import mybir
from concourse import bass_utils

NCORES = 8
DIM = 512
HALF = 256          # per-core cols per batch
NITER = 3           # LAYERS + 1
LN_EPS = 1e-5

F32 = mybir.dt.float32
F32R = mybir.dt.float32r
BF16 = mybir.dt.bfloat16
I32 = mybir.dt.int32
I16 = mybir.dt.int16
I8 = mybir.dt.int8
FP8 = mybir.dt.float8e4
PM = mybir.MatmulPerfMode
AF = mybir.ActivationFunctionType
OP = mybir.AluOpType

# Schraudolph exp on DVE: fp8e4m3(2^t) bits ~= i8(t*8 + 8*(7-sigma)),
# computed as one f32 tensor_scalar (mult, add) with i8 output dtype
# (DVE converts round-to-nearest).  t = raw_score * 0.125 * log2(e).
_LOG2E = 1.4426950408889634
SCH_SIGMA = 0.0430
SCH_A8 = 0.125 * _LOG2E * 8.0
SCH_B8 = 8.0 * (7.0 - SCH_SIGMA)

# A2A per-batch shard layout (flat bf16 words per (src,dst) pair):
#   [0:16384)      Q^T slice  (64 of-rows, 256 cols)
#   [16384:32768)  K^T slice  (64 of-rows, 256 cols)
#   [32768:49152)  V slice    (2 t-chunks, 128 rows, 64 fv-cols)
SHARD = 49152


def _build_graph(nc):
    xt_in = nc.dram_tensor("xt", [DIM, 2 * HALF], F32R, kind="ExternalInput").ap()
    wq_in = nc.dram_tensor("wq", [DIM, DIM], F32R, kind="ExternalInput").ap()
    wk_in = nc.dram_tensor("wk", [DIM, DIM], F32R, kind="ExternalInput").ap()
    wv_in = nc.dram_tensor("wv", [DIM, DIM], F32R, kind="ExternalInput").ap()
    wo_in = nc.dram_tensor("wo", [DIM, DIM], F32R, kind="ExternalInput").ap()
    bq_in = nc.dram_tensor("bq", [128, 4], F32, kind="ExternalInput").ap()
    bk_in = nc.dram_tensor("bk", [128, 4], F32, kind="ExternalInput").ap()
    bo_in = nc.dram_tensor("bo", [128, 4], F32, kind="ExternalInput").ap()
    bv_in = nc.dram_tensor("bv", [1, DIM], F32R, kind="ExternalInput").ap()
    lng_in = nc.dram_tensor("lng", [128, 4], F32, kind="ExternalInput").ap()
    lnb_in = nc.dram_tensor("lnb", [128, 4], F32, kind="ExternalInput").ap()
    ones_in = nc.dram_tensor("ones", [128, 128], F32R, kind="ExternalInput").ap()
    ones3_in = nc.dram_tensor("ones3", [128, 16, 1], BF16, kind="ExternalInput").ap()
    out_d = nc.dram_tensor("out", [DIM, 2 * HALF], F32R, kind="ExternalOutput").ap()

    groups = [list(range(NCORES))]

    from contextlib import ExitStack
    with tile.TileContext(nc) as tc, ExitStack() as ctx:
        const = ctx.enter_context(tc.tile_pool(name="const", bufs=1))
        act = ctx.enter_context(tc.tile_pool(name="act", bufs=1))
        qkv = ctx.enter_context(tc.tile_pool(name="qkv", bufs=1))
        gath = ctx.enter_context(tc.tile_pool(name="gath", bufs=2))
        epool = ctx.enter_context(tc.tile_pool(name="epool", bufs=3))
        small = ctx.enter_context(tc.tile_pool(name="small", bufs=1))
        dram = ctx.enter_context(tc.tile_pool(name="dram", bufs=1, space="DRAM"))
        s_psum = ctx.enter_context(tc.tile_pool(name="s_psum", bufs=2, space="PSUM"))
        o_psum = ctx.enter_context(tc.tile_pool(name="o_psum", bufs=2, space="PSUM"))
        p_psum = ctx.enter_context(tc.tile_pool(name="p_psum", bufs=1, space="PSUM"))
        if True:
            # ---- constants to SBUF ----
            def load_w(ap_in, nm):
                t = const.tile([128, 4, DIM], F32R, name=nm, tag=nm)
                nc.sync.dma_start(out=t, in_=ap_in.rearrange("(c p) f -> p c f", p=128))
                return t

            wq, wk, wv, wo = (load_w(wq_in, "wqt"), load_w(wk_in, "wkt"),
                              load_w(wv_in, "wvt"), load_w(wo_in, "wot"))
            bq = const.tile([128, 4], F32)
            bk = const.tile([128, 4], F32)
            bo = const.tile([128, 4], F32)
            lng = const.tile([128, 4], F32)
            lnb = const.tile([128, 4], F32)
            for t, a in ((bq, bq_in), (bk, bk_in), (bo, bo_in), (lng, lng_in), (lnb, lnb_in)):
                nc.sync.dma_start(out=t, in_=a)
            bv = const.tile([1, DIM], F32R)
            nc.sync.dma_start(out=bv, in_=bv_in)
            ones = const.tile([128, 128], F32R)
            nc.sync.dma_start(out=ones, in_=ones_in)
            ones3 = const.tile([128, 16, 1], BF16)
            nc.sync.dma_start(out=ones3, in_=ones3_in)
            eps_t = const.tile([1, 1], F32)
            nc.vector.memset(eps_t, LN_EPS)
            magic = const.tile([1, HALF], I32)
            nc.vector.memset(magic, 0x5F3759DF)

            # initial activation, as halves
            x0h = []
            for b in range(2):
                xb = act.tile([128, 4, HALF], F32R, tag=f"x0h{b}", name=f"x0h{b}")
                nc.sync.dma_start(
                    out=xb,
                    in_=xt_in.rearrange("(c p) f -> p c f", p=128)[:, :, 256 * b:256 * (b + 1)])
                x0h.append(xb)

            # DRAM bounce buffers
            sendb = [dram.tile([NCORES, SHARD], BF16, tag=f"send{b}",
                               name=f"send{b}") for b in range(2)]
            recvb = [dram.tile([NCORES, SHARD], BF16, tag=f"recv{b}",
                               name=f"recv{b}") for b in range(2)]
            rs_d = dram.tile([1, 512], F32, tag="rs_d", name="rs_d", bufs=2)
            stat_d = [dram.tile([2, HALF], F32, tag=f"stat{b}",
                                name=f"stat{b}", bufs=2) for b in range(2)]

            def proj_T_half(x_b, w, btile, tag, odt=F32R):
                """(128,4,HALF) <- relu(w^T x_b + bias), transposed output.
                Evacuation on ACT (relu+bias in one activation) to keep the
                DVE free for exp/LN work."""
                out = qkv.tile([128, 4, HALF], odt, tag=tag, name=tag)
                for pair in range(2):
                    ps = p_psum.tile([128, 2, HALF], F32, tag=f"p{pair}", name=f"ps_{tag}")
                    for i in range(2):
                        ofc = 2 * pair + i
                        for ifc in range(4):
                            nc.tensor.matmul(
                                ps[:, i, :],
                                w[:, ifc, 128 * ofc:128 * (ofc + 1)],
                                x_b[:, ifc, :],
                                start=(ifc == 0), stop=(ifc == 3))
                        nc.scalar.activation(
                            out[:, ofc, :], ps[:, i, :], AF.Relu,
                            bias=btile[:, ofc:ofc + 1])
                return out

            def proj_V_half(x_b, tag):
                """(128,2,DIM) bf16 <- relu(x_b^T wv + bv), natural layout."""
                out = qkv.tile([128, 2, DIM], BF16, tag=tag, name=tag)
                for tch in range(2):
                    ps = p_psum.tile([128, DIM], F32, tag="p", name=f"ps_{tag}{tch}")
                    for ifc in range(4):
                        nc.tensor.matmul(
                            ps,
                            x_b[:, ifc, 128 * tch:128 * (tch + 1)],
                            wv[:, ifc, :],
                            start=(ifc == 0), stop=False)
                    nc.tensor.matmul(
                        ps, ones[0:1, :], bv, start=False, stop=True)
                    nc.vector.tensor_scalar(
                        out=out[:, tch, :], in0=ps,
                        scalar1=0.0, scalar2=None, op0=OP.max)
                return out

            def make_exchange(b, xb):
                """qkv projection + pack + A2A + unpack as a list of small
                emission pieces, to be interleaved into the following
                attention so no engine FIFO gets a long head-of-line block.
                Tiles are allocated eagerly; instructions emit when pieces
                are invoked (in list order)."""
                qt_b = qkv.tile([128, 4, HALF], BF16, tag=f"qt{b}", name=f"qt{b}")
                kt_b = qkv.tile([128, 4, HALF], BF16, tag=f"kt{b}", name=f"kt{b}")
                v_b = qkv.tile([128, 2, DIM], BF16, tag=f"v{b}", name=f"v{b}")
                # qh: q^T features on partitions 0-63, duplicated on 64-127 so
                # row-tile T8 can stream its own copy.  kh: even kt-chunks on
                # partitions 0-63 (tile T0), odd kt-chunks on 64-127 (tile T8).
                qh = gath.tile([128, NCORES, 256], BF16, tag=f"qh{b}", name=f"qh{b}")
                kh = gath.tile([128, NCORES, 128], BF16, tag=f"kh{b}", name=f"kh{b}")
                vh = gath.tile([128, 16, 65], BF16, tag=f"vh{b}", name=f"vh{b}")
                vh8 = gath.tile([128, 8, 2, 80], FP8, tag=f"vh8{b}",
                                name=f"vh8{b}")
                sb, rb = sendb[b], recvb[b]
                pieces = []

                # mm and evac are SEPARATE pieces: the evac (DVE) is emitted
                # ~2 slots after its matmuls so it never head-of-line blocks
                # the exp stream while waiting on the PE.
                def qk_mm(w, out, pair, tag):
                    ps = p_psum.tile([128, 2, HALF], F32, tag=f"p{pair % 2}",
                                     name=f"ps_{tag}{pair}")

                    def f():
                        for i in range(2):
                            ofc = 2 * pair + i
                            for ifc in range(4):
                                nc.tensor.matmul(
                                    ps[:, i, :],
                                    w[:, ifc, 128 * ofc:128 * (ofc + 1)],
                                    xb[:, ifc, :],
                                    start=(ifc == 0), stop=(ifc == 3))
                    return f, ps

                def qk_evac(ps, btile, out, pair):
                    def f():
                        for i in range(2):
                            ofc = 2 * pair + i
                            nc.vector.tensor_scalar(
                                out=out[:, ofc, :], in0=ps[:, i, :],
                                scalar1=btile[:, ofc:ofc + 1], scalar2=0.0,
                                op0=OP.add, op1=OP.max)
                    return f

                def v_mm(tch):
                    ps = p_psum.tile([128, DIM], F32, tag=f"p{tch}",
                                     name=f"ps_v{b}{tch}")

                    def f():
                        for ifc in range(4):
                            nc.tensor.matmul(
                                ps,
                                xb[:, ifc, 128 * tch:128 * (tch + 1)],
                                wv[:, ifc, :],
                                start=(ifc == 0), stop=False)
                        nc.tensor.matmul(
                            ps, ones[0:1, :], bv, start=False, stop=True)
                    return f, ps

                def v_evac(ps, tch):
                    def f():
                        nc.vector.tensor_scalar(
                            out=v_b[:, tch, :], in0=ps,
                            scalar1=0.0, scalar2=None, op0=OP.max)
                    return f

                def pack_piece(which):
                    # dst d = 2*cq + pb owns q/k feature rows [64d, 64d+64) =
                    # ofc chunk cq, partition block pb.  One strided DMA per
                    # (tensor, pb) instead of one per destination.
                    def f():
                        if which == 0:
                            for pb in range(2):
                                nc.gpsimd.dma_start(
                                    out=sb[:, 0:16384].rearrange(
                                        "(cq pb) (r c) -> pb r cq c",
                                        pb=2, r=64, c=256)[pb],
                                    in_=qt_b[64 * pb:64 * pb + 64, :, :])
                        elif which == 1:
                            for pb in range(2):
                                nc.gpsimd.dma_start(
                                    out=sb[:, 16384:32768].rearrange(
                                        "(cq pb) (r c) -> pb r cq c",
                                        pb=2, r=64, c=256)[pb],
                                    in_=kt_b[64 * pb:64 * pb + 64, :, :])
                        else:
                            for tch in range(2):
                                nc.gpsimd.dma_start(
                                    out=sb[:, 32768:49152].rearrange(
                                        "d (tc p j) -> tc p d j",
                                        tc=2, p=128)[tch],
                                    in_=v_b[:, tch, :].rearrange(
                                        "p (d j) -> p d j", d=8))
                    return f

                def coll_piece():
                    nc.gpsimd.collective_compute(
                        "AllToAll", OP.bypass, replica_groups=groups,
                        ins=[sb.opt()], outs=[rb.opt()])

                def unpack_piece():
                    qsrc = rb[:, 0:16384].rearrange("s (r c) -> r s c", r=64)
                    nc.gpsimd.dma_start(out=qh[0:64], in_=qsrc)
                    nc.gpsimd.dma_start(out=qh[64:128], in_=qsrc)
                    ksrc = rb[:, 16384:32768].rearrange("s (r c) -> r s c", r=64)
                    nc.gpsimd.dma_start(out=kh[0:64], in_=ksrc[:, :, 0:128])
                    nc.gpsimd.dma_start(out=kh[64:128], in_=ksrc[:, :, 128:256])
                    for tc2 in range(2):
                        nc.gpsimd.dma_start(
                            out=vh[:, tc2::2, 0:64],
                            in_=rb[:, 32768 + 8192 * tc2:32768 + 8192 * (tc2 + 1)]
                                .rearrange("s (p j) -> p s j", p=128))
                    nc.gpsimd.dma_start(out=vh[:, :, 64:65], in_=ones3)

                def vconv_piece():
                    # bf16 -> fp8 with DoubleRow pair layout [pr, ko, 80-pad]
                    nc.vector.tensor_copy(
                        vh8[:, :, :, 0:65],
                        vh.rearrange("p (pr ko) f -> p pr ko f", ko=2))

                q0f, q0ps = qk_mm(wq, qt_b, 0, f"qt{b}")
                q1f, q1ps = qk_mm(wq, qt_b, 1, f"qt{b}")
                k0f, k0ps = qk_mm(wk, kt_b, 0, f"kt{b}")
                k1f, k1ps = qk_mm(wk, kt_b, 1, f"kt{b}")
                v0f, v0ps = v_mm(0)
                v1f, v1ps = v_mm(1)
                pieces += [
                    q0f, q1f,
                    qk_evac(q0ps, bq, qt_b, 0), k0f,
                    qk_evac(q1ps, bq, qt_b, 1), k1f,
                    qk_evac(k0ps, bk, kt_b, 0), v0f,
                    qk_evac(k1ps, bk, kt_b, 1), v1f,
                    v_evac(v0ps, 0), v_evac(v1ps, 1),
                    pack_piece(0), pack_piece(1), pack_piece(2),
                    coll_piece, unpack_piece, vconv_piece,
                ]
                return pieces, (qh, kh, vh8)

            def attention_half(b, qh, kh, vh, pieces=None):
                """(128,4,HALF) f32r Z^T for batch b (local Z rows).
                `pieces`: deque of emission closures (the next exchange)
                interleaved into the p-loop, ~one every other p-step."""
                pieces = pieces if pieces is not None else deque()
                z = qkv.tile([128, 4, HALF], F32R, tag=f"z{b}", name=f"z{b}")

                def norm_a(ops, j):
                    # rsum copy on ACT (PSUM-near), recip + broadcast kicked off
                    rsum = small.tile([1, 512], F32, tag="rsum", bufs=2, name="rsum")
                    nc.scalar.activation(rsum, ops[64:65, :], AF.Copy)
                    recip = small.tile([1, 512], F32, tag="recip", bufs=2,
                                       name="recip")
                    nc.vector.reciprocal_approx_fast(recip, rsum)
                    nc.sync.dma_start(out=rs_d, in_=recip)
                    rrep = small.tile([64, 512], F32, tag="rrep", bufs=2,
                                      name="rrep")
                    nc.sync.dma_start(
                        out=rrep, in_=rs_d.partition_broadcast(64)[:, 0, :])
                    return rrep

                def norm_b(ops, j, rrep):
                    o_v = ops[0:64, :].rearrange("f (r s) -> f s r", s=8)
                    r_v = rrep.rearrange("f (r s) -> f s r", s=8)
                    for q in range(2):
                        nc.vector.tensor_tensor(
                            out=z[64 * q:64 * (q + 1), :, 64 * j:64 * (j + 1)],
                            in0=o_v[:, q::2, :],
                            in1=r_v[:, q::2, :],
                            op=OP.mult)

                # j-pipelined: normalization of chunk j-1 is emitted inside
                # chunk j's p-loop so its DRAM-bounce broadcast hides behind
                # the exp stream instead of head-of-line blocking the DVE.
                pend = None
                for j in range(4):
                    ops = o_psum.tile([65, 512], F32, tag="o", name=f"ops{b}{j}")

                    def emit_av(ep, pp, last):
                        # fp8 DoubleRow: one matmul contracts BOTH kt chunks
                        # of the pair: out[m,n] = sum_ki sum_ko v[ki,ko,m]*e[ki,ko,n]
                        nc.tensor.matmul(ops, vh[:, pp, :, 0:65], ep,
                                         start=(pp == 0), stop=last,
                                         perf_mode=PM.DoubleRow)

                    # software-pipelined 2 deep: AV for step p-2 is emitted
                    # AFTER the scores of step p, so the PE FIFO never stalls
                    # on the (pair-granular) exp.
                    prevs = deque()
                    for p in range(8):
                        sps = s_psum.tile([128, 2, 512], F32, tag="s",
                                          name=f"sps{b}{j}{p}")
                        # concurrent row tiles: T0 (even chunk), T8 (odd chunk)
                        nc.tensor.matmul(
                            sps[:, 0, :], kh[0:64, p, :],
                            qh[0:64, 2 * j:2 * j + 2, :], start=True, stop=True)
                        nc.tensor.matmul(
                            sps[:, 1, :], kh[64:128, p, :],
                            qh[64:128, 2 * j:2 * j + 2, :], start=True, stop=True)
                        if len(prevs) == 2:
                            emit_av(*prevs.popleft(), last=False)
                        e = epool.tile([128, 2, 512], FP8, tag="e", name=f"e{b}{j}{p}")
                        # exp at pair granularity in fp8, split 5:3 between
                        # ACT (direct Exp -> fp8) and DVE (schraudolph i8
                        # bit-trick); one instruction covers both banks.
                        if p in (1, 3, 5):
                            nc.vector.tensor_scalar(
                                out=e.bitcast(I8), in0=sps,
                                scalar1=SCH_A8, scalar2=SCH_B8,
                                op0=OP.mult, op1=OP.add)
                        else:
                            nc.scalar.activation(e, sps, AF.Exp, scale=0.125)
                        prevs.append((e, p))
                        if p == 1 and pend is not None:
                            pend = pend + (norm_a(*pend),)
                        if p % 2 == 1:
                            for _ in range(3):
                                if pieces:
                                    pieces.popleft()()
                    while prevs:
                        ep, pp = prevs.popleft()
                        emit_av(ep, pp, last=(not prevs))
                    if pend is not None:
                        norm_b(*pend)
                    pend = (ops, j)
                pend = pend + (norm_a(*pend),)
                norm_b(*pend)
                return z

            def ln_half(x_b, b, resid=None):
                """LN over features (partitions) on one batch-half."""
                if resid is not None:
                    xr = act.tile([128, 4, HALF], F32R, tag=f"xr{b}", name=f"xr{b}")
                    nc.vector.tensor_tensor(out=xr, in0=x_b, in1=resid, op=OP.add)
                    x_b = xr
                x2 = act.tile([128, 4, HALF], F32R, tag=f"x2{b}", name=f"x2{b}")
                nc.vector.tensor_tensor(out=x2, in0=x_b, in1=x_b, op=OP.mult)
                mu_ps = p_psum.tile([1, HALF], F32, tag="p0", name=f"lnmu{b}")
                for ifc in range(4):
                    nc.tensor.matmul(mu_ps, ones[:, 0:1], x_b[:, ifc, :],
                                     start=(ifc == 0), stop=(ifc == 3))
                mu = small.tile([1, HALF], F32, tag="mu", bufs=2, name="mu")
                nc.vector.tensor_scalar(out=mu, in0=mu_ps, scalar1=1.0 / DIM,
                                        scalar2=None, op0=OP.mult)
                m2_ps = p_psum.tile([1, HALF], F32, tag="p0", name=f"lnm2{b}")
                for ifc in range(4):
                    nc.tensor.matmul(m2_ps, ones[:, 1:2], x2[:, ifc, :],
                                     start=(ifc == 0), stop=(ifc == 3))
                ex2 = small.tile([1, HALF], F32, tag="ex2", bufs=2, name="ex2")
                nc.vector.tensor_scalar(out=ex2, in0=m2_ps, scalar1=1.0 / DIM,
                                        scalar2=None, op0=OP.mult)
                m2 = small.tile([1, HALF], F32, tag="m2", bufs=2, name="m2")
                nc.vector.tensor_tensor(out=m2, in0=mu, in1=mu, op=OP.mult)
                nc.vector.tensor_tensor(out=ex2, in0=ex2, in1=m2, op=OP.subtract)
                nc.vector.tensor_scalar(out=ex2, in0=ex2, scalar1=LN_EPS,
                                        scalar2=None, op0=OP.add)
                # rstd = rsqrt(var+eps) fully on DVE: bit-trick seed + 2 Newton
                # steps (keeps ACT on the exp table set -> no table reloads).
                sd = small.tile([1, HALF], F32, tag="sd", bufs=2, name="sd")
                sdi = sd.bitcast(I32)
                nc.vector.tensor_scalar(out=sdi, in0=ex2.bitcast(I32), scalar1=1,
                                        scalar2=None, op0=OP.logical_shift_right)
                nc.vector.tensor_tensor(out=sdi, in0=magic, in1=sdi, op=OP.subtract)
                for _ in range(2):
                    nc.vector.tensor_tensor(out=m2, in0=ex2, in1=sd, op=OP.mult)
                    nc.vector.tensor_tensor(out=m2, in0=m2, in1=sd, op=OP.mult)
                    nc.vector.tensor_scalar(out=m2, in0=m2, scalar1=-0.5,
                                            scalar2=1.5, op0=OP.mult, op1=OP.add)
                    nc.vector.tensor_tensor(out=sd, in0=sd, in1=m2, op=OP.mult)
                nc.vector.tensor_tensor(out=mu, in0=mu, in1=sd, op=OP.mult)
                # broadcast rstd & mu*rstd across partitions via DRAM bounce
                nc.sync.dma_start(out=stat_d[b][0:1, :], in_=sd)
                nc.sync.dma_start(out=stat_d[b][1:2, :], in_=mu)
                srep = small.tile([128, 2, HALF], F32, tag=f"srep{b}", bufs=2,
                                  name=f"srep{b}")
                nc.sync.dma_start(out=srep, in_=stat_d[b].partition_broadcast(128))
                out = act.tile([128, 4, HALF], F32R, tag=f"lnout{b}", bufs=3,
                               name=f"lnout{b}")
                for ifc in range(4):
                    t1 = small.tile([128, HALF], F32, tag=f"t1{b}", bufs=2,
                                    name=f"t1{b}")
                    nc.vector.tensor_tensor(out=t1, in0=x_b[:, ifc, :],
                                            in1=srep[:, 0, :], op=OP.mult)
                    nc.vector.tensor_tensor(out=t1, in0=t1, in1=srep[:, 1, :],
                                            op=OP.subtract)
                    nc.vector.tensor_scalar(
                        out=out[:, ifc, :], in0=t1,
                        scalar1=lng[:, ifc:ifc + 1], scalar2=lnb[:, ifc:ifc + 1],
                        op0=OP.mult, op1=OP.add)
                return out

            # Staged pipeline over (sublayer, batch) stages.  The exchange
            # (qkv proj + pack + A2A + unpack) for stage (s+1, b) is created
            # right after stage (s, b)'s LN and its pieces are interleaved
            # into the FOLLOWING stage's attention p-loop, so no engine FIFO
            # ever sees a multi-microsecond head-of-line block and the
            # collective flies under attention compute.
            NSUB = 2 * NITER
            cur_in = list(x0h)
            prev_in = [None, None]
            pk0, h0 = make_exchange(0, x0h[0])
            for f in pk0:
                f()
            pk1, h1 = make_exchange(1, x0h[1])
            for f in pk1:
                f()
            pend = [h0, h1]
            carry = deque()
            for s in range(NSUB):
                for b in range(2):
                    z_b = attention_half(b, *pend[b], pieces=carry)
                    while carry:
                        carry.popleft()()
                    y_b = proj_T_half(z_b, wo, bo, f"y{b}", odt=F32R)
                    resid = prev_in[b] if (s % 2 == 1) else None
                    xout = ln_half(y_b, b, resid=resid)
                    if s < NSUB - 1:
                        carry, pend[b] = make_exchange(b, xout)
                        carry = deque(carry)
                    else:
                        carry = deque()
                    prev_in[b] = cur_in[b]
                    cur_in[b] = xout
            init = cur_in

            for b in range(2):
                nc.sync.dma_start(
                    out=out_d.rearrange("(c p) f -> p c f", p=128)[:, :, 256 * b:256 * (b + 1)],
                    in_=init[b])
    return nc


_NC_CACHE = None


def _get_nc():
    global _NC_CACHE
    if _NC_CACHE is None:
        nc = bacc.Bacc("TRN2", target_bir_lowering=False, debug=False,
                       num_devices=NCORES)
        _build_graph(nc)
        nc.compile()
        _NC_CACHE = nc
    return _NC_CACHE


def kernel(encoder_inputs, Wq, bq, Wk, bk, Wv, bv, Wo, bo, ln_g, ln_b,
           _trace=False, _trace_kwargs=None):
    x = np.asarray(encoder_inputs, dtype=np.float32)
    consts = {
        "wq": np.ascontiguousarray(np.asarray(Wq, np.float32)),
        "wk": np.ascontiguousarray(np.asarray(Wk, np.float32)),
        "wv": np.ascontiguousarray(np.asarray(Wv, np.float32)),
        "wo": np.ascontiguousarray(np.asarray(Wo, np.float32)),
        "bq": np.ascontiguousarray(np.asarray(bq, np.float32).reshape(4, 128).T),
        "bk": np.ascontiguousarray(np.asarray(bk, np.float32).reshape(4, 128).T),
        "bo": np.ascontiguousarray(np.asarray(bo, np.float32).reshape(4, 128).T),
        "bv": np.asarray(bv, np.float32).reshape(1, DIM),
        "lng": np.ascontiguousarray(np.asarray(ln_g, np.float32).reshape(4, 128).T),
        "lnb": np.ascontiguousarray(np.asarray(ln_b, np.float32).reshape(4, 128).T),
        "ones": np.ones((128, 128), np.float32),
        "ones3": np.ones((128, 16, 1), ml_dtypes.bfloat16),
    }
    in_maps = []
    for c in range(NCORES):
        xt = np.concatenate([x[0, 256 * c:256 * (c + 1)].T,
                             x[1, 256 * c:256 * (c + 1)].T], axis=1)
        in_maps.append({"xt": np.ascontiguousarray(xt), **consts})

    nc = _get_nc()
    res = bass_utils.run_bass_kernel_spmd(
        nc, in_maps, core_ids=list(range(NCORES)),
        trace=_trace, **(_trace_kwargs or {}))

    out = np.zeros((2, 2048, DIM), np.float32)
    for c in range(NCORES):
        r = res.results[c]["out"]
        out[0, 256 * c:256 * (c + 1)] = r[:, :256].T
        out[1, 256 * c:256 * (c + 1)] = r[:, 256:].T
    if _trace:
        kernel._last_results = res
    return out

